# revision 65
# baseline (speedup 1.0000x reference)
"""RWKV6 (nn_ExtendedMemory) Trainium2 kernel — 8 NeuronCores, v3.

Sharding: core c -> batch c//2, tensor-parallel half c%2 (8 of 16 heads,
half of the FFN columns). Two pairwise bf16 AllReduces per layer window
(Wo output and cWv output) via collective_compute.

v3 host/runtime design (supersedes v2's tunnel-upload optimizations):
  - _Runner AOT-compiles the module ONCE (jit(shard_map).lower().compile()
    via bass2jax's fast-dispatch path) and keeps all inputs device-resident
    across calls; a steady-state call is pure dispatch + NEFF execution.
  - Inputs are consolidated into 4 operands (partition_id, x, wblob, vblob)
    because the axon PJRT runtime costs ~40us per operand per call.
    wblob: all matmul weights as [229,128,512] bf16 bricks (+ident/blkdiag
    constants); vblob: all f32 vectors/constants packed as [128, VC].
  - No input AllGathers / output ReduceScatter: each core receives its full
    half-set of weights and full x, and writes the full (pair-identical) y.
  - LN/elementwise chain runs bf16 (PE matmuls bf16); the residual stream
    stays f32 in DRAM with a bf16 mirror for LN input. Batched 3D vector
    ops (token shift, LN normalize via stride-0 broadcast APs, residual
    applies) and ACT-engine offload (PSUM evacuation, relu, delta casts)
    keep DVE instruction count down.
"""

import os
import numpy as np
import ml_dtypes

import concourse.bass as bass
import concourse.mybir as mybir
import concourse.tile as tile
from concourse.bass_utils import run_bass_kernel_spmd

dt = mybir.dt
Alu = mybir.AluOpType
Act = mybir.ActivationFunctionType
BF16 = ml_dtypes.bfloat16
F8 = ml_dtypes.float8_e4m3
SCW = 1.0             # weight pre-scale (1.0 = bf16 blob, no scaling)
ISCW = 1.0 / SCW

L, D, HS, E, ED, FE = 2, 1024, 64, 32, 64, 3
H = D // HS            # 16 heads total
B = 4
EPS = 1e-5
NH = H // 2            # 8 heads per core
CH = NH * HS           # 512 channels per core
DT = D // 128          # 8 D-tiles
CT = CH // 128         # 4 chan-tiles per core
FT = FE * D // 2 // 128  # 12 ffn-tiles per core
CK = 128               # wkv chunk
WIN = 256              # token window

PAIRS = [[0, 1], [2, 3], [4, 5], [6, 7]]
if os.environ.get("KSELF"):      # timing probe: no inter-core sync
    PAIRS = [[c] for c in range(8)]
COLS = [[0, 2, 4, 6], [1, 3, 5, 7]]

# big-weight brick blob: [nbricks, 128, 512] bf16 per half-set
BPL = 114              # bricks per layer (104 big + 9 small + 1 pad)
NBR = BPL * L          # 228 bricks per half-set
SHARD = NBR // 4       # 57 bricks shipped per core


def _woff(l):
    o = BPL * l
    return dict(Wr=o, Wk=o + 8, Wv=o + 16, Wg=o + 24, Wo=o + 32,
                cWk=o + 40, cWv=o + 64, cWr=o + 88,
                tm_w1=o + 104, tm_w2=o + 107, td_w1=o + 111, td_w2=o + 112)


IDBRICK = NBR          # extra brick: ident [*,0:128], blkdiag [*,128:130]


def _vlayout(skip_gn_affine, skip_ln_w):
    """column layout of the packed f32 vector blob [128, VC]"""
    off = {}
    c = 0
    off['maskstr'] = c
    c += 128
    for l in range(L):
        for n in ["x_maa", "w_maa", "k_maa", "v_maa", "r_maa", "g_maa",
                  "ck_maa", "cr_maa"]:
            off[(n, l)] = c
            c += DT
        off[('tdb', l)] = c
        c += CT
        off[('u', l)] = c
        c += CT
        if not skip_ln_w:
            off[('ln1', l)] = c
            c += DT
            off[('ln2', l)] = c
            c += DT
        if not skip_gn_affine:
            off[('gnw', l)] = c
            c += CH
            off[('gnb', l)] = c
            c += CH
    if not skip_ln_w:
        off['lnf'] = c
        c += DT
    return off, c


TC = tile.TileContext


_wsplit_counter = [0]


def _split_sync_waits(nc, scratch=None, max_waits=1):
    """walrus in this container rejects >1 sync wait per instruction.

    For single-queue engines (PE/DVE/ACT/SP) excess waits move onto
    same-engine standalone EventSemaphore instructions placed immediately
    before the owner (engine streams are strict FIFO, so this is
    equivalent). GpSimd fans instructions across 8 Q7 queues, so a
    standalone wait there guards nothing — instead its waits are relayed:
    SP waits each semaphore (EVSEM chain), then bumps a scratch semaphore
    that the Pool instruction waits on (its single allowed wait)."""
    if scratch is None:
        scratch = nc.alloc_semaphore("wsplit_scratch")
    scratch_count = [0]

    def evsem(engine, waits, updates=()):
        _wsplit_counter[0] += 1
        ev = mybir.InstEventSemaphore(
            name=f"I-wsplit-{_wsplit_counter[0]}", ins=[], outs=[])
        ev.engine = engine
        ev.sync_info = mybir.SyncInfo(on_wait=list(waits),
                                      on_update=list(updates))
        return ev

    sp = mybir.EngineType.Activation
    for f in nc.m.functions:
        for bb in f.blocks:
            out = []
            changed = False
            for inst in bb.instructions:
                si = inst.sync_info
                if si is not None and len(si.on_wait) > max_waits:
                    waits = list(si.on_wait)
                    changed = True
                    if inst.engine == mybir.EngineType.Pool:
                        for wv in waits:
                            out.append(evsem(sp, [wv]))
                        scratch_count[0] += 1
                        out.append(evsem(sp, [], [mybir.SyncUpdate(
                            sync_type="semaphore", id=scratch.num,
                            update_mode="sem-inc", update_value=1)]))
                        keep = [mybir.SyncWait(
                            sync_type="semaphore", id=scratch.num,
                            wait_mode="sem-ge-imm",
                            wait_value=scratch_count[0])]
                    else:
                        extra, keep = waits[:-max_waits], waits[-max_waits:]
                        while extra:
                            chunk, extra = (extra[:max_waits],
                                            extra[max_waits:])
                            out.append(evsem(inst.engine, chunk))
                    inst.sync_info = mybir.SyncInfo(
                        on_wait=keep, on_update=list(si.on_update))
                out.append(inst)
            if changed:
                bb.instructions = out


def build(nc, T, skip_gn_affine, skip_ln_w, split_waits=True):
    W = min(WIN, T)
    assert T % W == 0
    NW = T // W
    NCH = W // CK or 1
    assert W % CK == 0
    WC = min(2 * WIN, T)   # wider windows for the final LN
    assert T % WC == 0
    NWC = T // WC

    f32, bf = dt.float32, dt.bfloat16

    def din(name, shape, d=f32):
        return nc.dram_tensor(name, shape, d, kind="ExternalInput")

    x_d = din("x", [DT, 128, T], bf)
    wb_d = din("wblob", [NBR + 1, 128, 512], bf)
    y_d = nc.dram_tensor("y", [DT, 128, T], bf, kind="ExternalOutput")

    OFFV, VC = _vlayout(skip_gn_affine, skip_ln_w)
    vb_d = din("vblob", [128, VC])

    # reserved before the TileContext so Tile's allocator can't recycle it
    wsplit_sem = nc.alloc_semaphore("wsplit_scratch")
    nc.sync.sem_clear(wsplit_sem)

    with TC(nc) as tc:
        import contextlib
        ctx = contextlib.ExitStack()
        with ctx:
            const = ctx.enter_context(tc.tile_pool(name="const", bufs=1))
            dram = ctx.enter_context(tc.tile_pool(name="dramb", bufs=2, space="DRAM"))
            xrp = ctx.enter_context(tc.tile_pool(name="xrp", bufs=1, space="DRAM"))
            xres_t = xrp.tile([DT, 128, T], f32, tag="xres")
            xresb_t = xrp.tile([DT, 128, T], bf, tag="xresb")

            # weights arrive full per core (device-resident across calls),
            # x arrives full per core: no input AllGathers needed.
            wag_out = wb_d

            def xsrc_bf(phase, sl):
                """bf16 LN-input window [128, DT, W] source for a phase."""
                t = x_d if phase == 0 else xresb_t
                return t[:, :, sl].rearrange("j p w -> p j w")

            ident = const.tile([128, 128], bf)
            nc.sync.dma_start(out=ident, in_=wb_d[IDBRICK, :, 0:128])
            maskstr = const.tile([128, 128], f32)   # keep j < i over [j, i]
            nc.sync.dma_start(
                out=maskstr,
                in_=vb_d[:, OFFV['maskstr']:OFFV['maskstr'] + 128])
            blkdiag = const.tile([128, 2], bf)      # col a = partitions 64a..
            nc.sync.dma_start(out=blkdiag, in_=wb_d[IDBRICK, :, 128:130])
            ones_bf = const.tile([128, 1], bf)
            nc.vector.memset(ones_bf, 1.0)
            ones_f = const.tile([128, 1], f32)
            nc.vector.memset(ones_f, 1.0)
            ones_rowb = const.tile([1, 128], bf)
            nc.vector.memset(ones_rowb, 1.0)
            zerosCK = const.tile([128, CK], f32)
            nc.vector.memset(zerosCK, 0.0)
            epst = const.tile([128, 1], f32)
            nc.vector.memset(epst, EPS)

            def jbc(t):
                """broadcast a [128, W] tile over the DT axis (stride-0 AP)"""
                a = t[:, :]
                return bass.AP(tensor=a.tensor, offset=a.offset,
                               ap=[list(a.ap[0]), [0, DT], list(a.ap[1])])

            def layer_norm(pool, ps_pool, lnw_t, xsrc, W=None):
                """LN over channels. xsrc: [128, DT, W] SBUF bf16 window.
                Returns xln [128, DT, W] bf16."""
                if W is None:
                    W = WIN if T >= WIN else T
                W = xsrc.shape[2]
                ps = ps_pool.tile([128, W], f32, tag="mm")
                ps_sq = ps_pool.tile([128, W], f32, tag="mm")
                sqa = pool.tile([128, DT, W], bf, tag="ln_sqa")
                nc.scalar.activation(sqa, xsrc, Act.Square)
                for j in range(DT):
                    nc.tensor.matmul(ps[0:1, :], lhsT=ones_bf,
                                     rhs=xsrc[:, j, :],
                                     start=(j == 0), stop=(j == DT - 1))
                    nc.tensor.matmul(ps_sq[0:1, :], lhsT=ones_bf,
                                     rhs=sqa[:, j, :],
                                     start=(j == 0), stop=(j == DT - 1))
                mu = pool.tile([1, W], f32, tag="ln_mu")
                nc.vector.tensor_scalar_mul(mu, ps[0:1, :], 1.0 / D)
                mub = pool.tile([1, W], bf, tag="ln_mub")
                nc.vector.tensor_copy(mub, mu)
                musq = pool.tile([1, W], f32, tag="ln_musq")
                nc.vector.tensor_mul(musq, mu, mu)
                var = pool.tile([1, W], f32, tag="ln_var")
                nc.vector.scalar_tensor_tensor(out=var, in0=ps_sq[0:1, :],
                                               scalar=1.0 / D, in1=musq,
                                               op0=Alu.mult, op1=Alu.subtract)
                sd = pool.tile([1, W], f32, tag="ln_sd")
                nc.scalar.activation(sd, var, Act.Sqrt, bias=epst[0:1], scale=1.0)
                rstdb = pool.tile([1, W], bf, tag="ln_rstdb")
                with nc.allow_low_precision(reason="bf16 rstd is plenty for LN"):
                    nc.vector.reciprocal(rstdb, sd)
                ps_b = ps_pool.tile([128, W], f32, tag="mm")
                nc.tensor.matmul(ps_b, lhsT=ones_rowb, rhs=mub, start=True,
                                 stop=True)
                mur = pool.tile([128, W], bf, tag="ln_mur")
                nc.vector.tensor_copy(mur, ps_b)
                ps_b2 = ps_pool.tile([128, W], f32, tag="mm")
                nc.tensor.matmul(ps_b2, lhsT=ones_rowb, rhs=rstdb, start=True,
                                 stop=True)
                rstdr = pool.tile([128, W], bf, tag="ln_rstdr")
                nc.vector.tensor_copy(rstdr, ps_b2)
                xln = pool.tile([128, DT, W], bf, tag="ln_out")
                if skip_ln_w:
                    nc.vector.tensor_sub(xln, xsrc, jbc(mur))
                    nc.vector.tensor_mul(xln, xln, jbc(rstdr))
                else:
                    tmp = pool.tile([128, W], bf, tag="ln_tmp")
                    for j in range(DT):
                        nc.vector.tensor_sub(tmp, xsrc[:, j, :], mur)
                        nc.vector.scalar_tensor_tensor(
                            out=xln[:, j, :], in0=tmp, scalar=lnw_t[:, j:j + 1],
                            in1=rstdr, op0=Alu.mult, op1=Alu.mult)
                return xln

            def token_shift(pool, xln, xln_prev, w):
                sx = pool.tile([128, DT, W], bf, tag="sx")
                nc.vector.tensor_sub(sx[:, :, 1:W], xln[:, :, 0:W - 1],
                                     xln[:, :, 1:W])
                if w == 0:
                    nc.vector.tensor_scalar_mul(sx[:, :, 0:1],
                                                xln[:, :, 0:1], -1.0)
                else:
                    nc.vector.tensor_sub(sx[:, :, 0:1],
                                         xln_prev[:, :, W - 1:W],
                                         xln[:, :, 0:1])
                return sx

            def mm_chain(ps, lhsT_f, rhs_f, nkt):
                for kt in range(nkt):
                    nc.tensor.matmul(ps, lhsT=lhsT_f(kt), rhs=rhs_f(kt),
                                     start=(kt == 0), stop=(kt == nkt - 1))

            # ================= layers =================
            _STOP = int(os.environ.get("KSTOP", "99"))
            _SUB = int(os.environ.get("KSUB", "99"))
            for l in range(L):
                if _STOP < 1 + 2 * l:
                    break
                phase_tm = 2 * l      # residual source phase id
                phase_cm = 2 * l + 1
                OFF = _woff(l)

                # ---------------- time mix ----------------
                with tc.tile_pool(name=f"wtm{l}", bufs=1) as wp, \
                     tc.tile_pool(name=f"vec{l}", bufs=1) as vp, \
                     tc.tile_pool(name=f"tma{l}", bufs=2) as pw, \
                     tc.tile_pool(name=f"tmb{l}", bufs=1) as pk, \
                     tc.tile_pool(name=f"tmc{l}", bufs=1) as pb, \
                     tc.tile_pool(name=f"tap{l}", bufs=1) as pa1, \
                     tc.tile_pool(name=f"st{l}", bufs=2) as sp, \
                     tc.tile_pool(name=f"psa{l}", bufs=2, space="PSUM") as psA, \
                     tc.tile_pool(name=f"psb{l}", bufs=1, space="PSUM") as psB, \
                     tc.tile_pool(name=f"pst{l}", bufs=1, space="PSUM") as psT:

                    w_r = wp.tile([128, DT, CH], bf, tag="w_r")
                    w_k = wp.tile([128, DT, CH], bf, tag="w_k")
                    w_v = wp.tile([128, DT, CH], bf, tag="w_v")
                    w_g = wp.tile([128, DT, CH], bf, tag="w_g")
                    for wt, nm in [(w_r, 'Wr'), (w_k, 'Wk'),
                                   (w_v, 'Wv'), (w_g, 'Wg')]:
                        nc.sync.dma_start(
                            out=wt,
                            in_=wag_out[OFF[nm]:OFF[nm] + DT]
                            .rearrange("n p c -> p n c"))
                    tmw1 = wp.tile([128, DT, 5 * E], bf, tag="tmw1")
                    tdw1 = wp.tile([128, DT, ED], bf, tag="tdw1")
                    for j in range(DT):
                        c0 = 160 * (j % 3)
                        nc.sync.dma_start(
                            out=tmw1[:, j, :],
                            in_=wag_out[OFF['tm_w1'] + j // 3, :, c0:c0 + 160])
                        nc.sync.dma_start(
                            out=tdw1[:, j, :],
                            in_=wag_out[OFF['td_w1'], :, 64 * j:64 * (j + 1)])
                    tmw2 = wp.tile([128, 2, D], bf, tag="tmw2")
                    for j in range(2):
                        for q in range(2):
                            nc.sync.dma_start(
                                out=tmw2[:, j, 512 * q:512 * (q + 1)],
                                in_=wag_out[OFF['tm_w2'] + 2 * j + q])
                    tdw2 = wp.tile([ED, CH], bf, tag="tdw2")
                    nc.sync.dma_start(out=tdw2, in_=wag_out[OFF['td_w2'], 0:ED, :])
                    wo = wp.tile([128, CT, D], bf, tag="wo")
                    nc.sync.dma_start(
                        out=wo.rearrange("p j (q c) -> p j q c", c=512),
                        in_=wag_out[OFF['Wo']:OFF['Wo'] + 2 * CT]
                        .rearrange("(j q) p c -> p j q c", q=2))

                    if skip_ln_w:
                        ln1w = None
                    else:
                        ln1w = vp.tile([128, DT], f32, tag="ln1w")
                        nc.sync.dma_start(
                            out=ln1w, in_=vb_d[:, OFFV[('ln1', l)]:
                                              OFFV[('ln1', l)] + DT])
                    maat = {}
                    for n in ["x_maa", "w_maa", "k_maa", "v_maa", "r_maa",
                              "g_maa"]:
                        maat[n] = vp.tile([128, DT], f32, tag=n, name=n)
                        nc.sync.dma_start(
                            out=maat[n],
                            in_=vb_d[:, OFFV[(n, l)]:OFFV[(n, l)] + DT])
                    tdb = vp.tile([128, CT], f32, tag="tdb")
                    nc.sync.dma_start(
                        out=tdb,
                        in_=vb_d[:, OFFV[('tdb', l)]:OFFV[('tdb', l)] + CT])
                    ut = vp.tile([128, CT], f32, tag="ut")
                    nc.sync.dma_start(
                        out=ut, in_=vb_d[:, OFFV[('u', l)]:OFFV[('u', l)] + CT])
                    if not skip_gn_affine:
                        gnw_r = vp.tile([128, CH], bf, tag="gnw_r")
                        nc.sync.dma_start(
                            out=gnw_r, in_=vb_d[:, OFFV[('gnw', l)]:
                                               OFFV[('gnw', l)] + CH])
                        gnb_r = vp.tile([128, CH], bf, tag="gnb_r")
                        nc.sync.dma_start(
                            out=gnb_r, in_=vb_d[:, OFFV[('gnb', l)]:
                                               OFFV[('gnb', l)] + CH])

                    S_cur = sp.tile([128, CT, HS], bf, tag="S")
                    nc.vector.memset(S_cur, 0.0)

                    def tm_apply(pend, last=False):
                        """apply window w-1's AllReduced delta to the residual
                        stream; issued one window late so compute never head-
                        of-line blocks on the collective. The final apply runs
                        on the (otherwise idle) Pool queue so the next phase's
                        DVE/ACT work isn't queued behind the last AR."""
                        bout_p, sl_p = pend
                        add_eng = nc.gpsimd if last else nc.vector
                        ar = pb.tile([128, DT, W], bf, tag="ar")
                        nc.sync.dma_start(out=ar, in_=bout_p)
                        xnew = pa1.tile([128, DT, W], f32, tag="xnew")
                        if phase_tm == 0:
                            xb = pa1.tile([128, DT, W], bf, tag="xb")
                            nc.sync.dma_start(
                                out=xb, in_=x_d[:, :, sl_p]
                                .rearrange("j p w -> p j w"))
                            add_eng.tensor_add(xnew, xb, ar)
                        else:
                            nc.sync.dma_start(
                                out=xnew, in_=xres_t[:, :, sl_p]
                                .rearrange("j p w -> p j w"))
                            add_eng.tensor_add(xnew, xnew, ar)
                        xnb = pa1.tile([128, DT, W], bf, tag="xnb")
                        if last:
                            nc.gpsimd.tensor_copy(xnb, xnew)
                        else:
                            nc.scalar.activation(xnb, xnew, Act.Identity)
                        nc.sync.dma_start(
                            out=xres_t[:, :, sl_p].rearrange("j p w -> p j w"),
                            in_=xnew)
                        nc.sync.dma_start(
                            out=xresb_t[:, :, sl_p].rearrange("j p w -> p j w"),
                            in_=xnb)

                    pend = None
                    xln_prev = None
                    for w in range(NW):
                        sl = slice(w * W, (w + 1) * W)
                        xwb = pw.tile([128, DT, W], bf, tag="xwb")
                        nc.sync.dma_start(out=xwb, in_=xsrc_bf(phase_tm, sl))
                        xln = layer_norm(pw, psA, ln1w, xwb)
                        xp_prev = xln_prev
                        xln_prev = xln
                        if _SUB < 2:
                            continue
                        sx = token_shift(pw, xln, xp_prev, w)

                        xxx = pw.tile([128, DT, W], bf, tag="xxx")
                        for j in range(DT):
                            nc.vector.scalar_tensor_tensor(
                                out=xxx[:, j, :], in0=sx[:, j, :],
                                scalar=maat["x_maa"][:, j:j + 1],
                                in1=xln[:, j, :], op0=Alu.mult, op1=Alu.add)

                        t320 = pw.tile([128, 2, W], bf, tag="t320")
                        for mt in range(2):
                            msz = min(128, 5 * E - mt * 128)
                            ps = psA.tile([128, W], f32, tag="mm")
                            mm_chain(ps[0:msz, :],
                                     lambda kt, mt=mt, msz=msz:
                                         tmw1[:, kt, mt * 128:mt * 128 + msz],
                                     lambda kt: xxx[:, kt, :], DT)
                            nc.scalar.activation(t320[0:msz, mt, :],
                                                 ps[0:msz, :], Act.Tanh)

                        bnames = ["w_maa", "k_maa", "v_maa", "r_maa", "g_maa"]
                        bx = {}
                        for f, n in enumerate(bnames):
                            bxf = pb.tile([128, DT, W], bf, tag=f"bx{f}")
                            poff = E * (f % 4)
                            fj = f // 4
                            s2a = pw.tile([128, DT, W], bf, tag="s2")
                            for mt in range(DT):
                                ps = psA.tile([128, W], f32, tag="mm")
                                nc.tensor.matmul(
                                    ps, lhsT=tmw2[poff:poff + E, fj,
                                                  mt * 128:(mt + 1) * 128],
                                    rhs=t320[poff:poff + E, fj, :],
                                    start=True, stop=True,
                                    tile_position=(poff, 0))
                                nc.vector.scalar_tensor_tensor(
                                    out=s2a[:, mt, :], in0=ps,
                                    scalar=maat[n][:, mt:mt + 1],
                                    in1=sx[:, mt, :], op0=Alu.add, op1=Alu.mult)
                            nc.vector.tensor_add(bxf, s2a, xln)
                            bx[n[0]] = bxf

                        if _SUB < 3:
                            continue
                        projs = {}
                        for n, wt, act in [("r", w_r, Act.Sigmoid),
                                           ("k", w_k, None), ("v", w_v, None),
                                           ("g", w_g, "silu")]:
                            out_t = pk.tile([128, CT, W], bf, tag=f"proj_{n}")
                            for mt in range(CT):
                                ps = psA.tile([128, W], f32, tag="mm")
                                mm_chain(ps,
                                         lambda kt, wt=wt, mt=mt:
                                             wt[:, kt, mt * 128:(mt + 1) * 128],
                                         lambda kt, n=n: bx[n][:, kt, :], DT)
                                if act is None:
                                    nc.vector.tensor_scalar_mul(
                                        out_t[:, mt, :], ps, ISCW)
                                elif act == "silu":
                                    sgt = pw.tile([128, W], bf, tag="sgt")
                                    nc.scalar.activation(sgt, ps, Act.Sigmoid,
                                                         scale=ISCW)
                                    nc.vector.tensor_mul(out_t[:, mt, :], sgt,
                                                         ps)
                                else:
                                    nc.scalar.activation(out_t[:, mt, :], ps,
                                                         act, scale=ISCW)
                            projs[n] = out_t

                        ps = psA.tile([128, W], f32, tag="mm")
                        mm_chain(ps[0:ED, :],
                                 lambda kt: tdw1[:, kt, :],
                                 lambda kt: bx["w"][:, kt, :], DT)
                        t64 = pw.tile([ED, W], bf, tag="t64")
                        nc.scalar.activation(t64, ps[0:ED, :], Act.Tanh)
                        wtotW = pw.tile([128, CT, NCH], f32, tag="wtotW")
                        rt_t = pk.tile([128, CT, W], bf, tag="rt_t")
                        kt_t = pk.tile([128, CT, W], bf, tag="kt_t")
                        kh_t = pk.tile([128, CT, W], bf, tag="kh_t")
                        rk_t = pk.tile([128, CT, W], bf, tag="rk_t")
                        e_all = pk.tile([128, CT, W], f32, tag="e_all")
                        P_all = pk.tile([128, CT, W], f32, tag="P_all")
                        for mt in range(CT):
                            ps2 = psA.tile([128, W], f32, tag="mm")
                            nc.tensor.matmul(ps2,
                                             lhsT=tdw2[:, mt * 128:(mt + 1) * 128],
                                             rhs=t64, start=True, stop=True)
                            nc.scalar.activation(e_all[:, mt, :], ps2, Act.Exp,
                                                 bias=tdb[:, mt:mt + 1],
                                                 scale=1.0)
                            # P = cumsum(e) within each chunk;  c_incl = -P
                            for c in range(NCH):
                                csl = slice(c * CK, (c + 1) * CK)
                                nc.vector.tensor_tensor_scan(
                                    out=P_all[:, mt, csl],
                                    data0=e_all[:, mt, csl],
                                    data1=zerosCK, initial=0.0,
                                    op0=Alu.add, op1=Alu.add)
                            nc.vector.scalar_tensor_tensor(
                                out=rk_t[:, mt, :], in0=projs["r"][:, mt, :],
                                scalar=ut[:, mt:mt + 1],
                                in1=projs["k"][:, mt, :],
                                op0=Alu.mult, op1=Alu.mult)
                        # batched decay tail over all mt at once
                        nc.vector.tensor_sub(e_all, P_all, e_all)  # := P - e
                        expA = pk.tile([128, CT, W], bf, tag="expA")
                        nc.scalar.activation(expA, e_all, Act.Exp, scale=-1.0)
                        expB = pk.tile([128, CT, W], bf, tag="expB")
                        nc.scalar.activation(expB, P_all, Act.Exp, scale=1.0)
                        pvv = P_all.rearrange("p m (c u) -> p (m c) u", u=CK)
                        nc.scalar.activation(
                            wtotW.rearrange("p m (c u) -> p (m c) u", u=1),
                            pvv[:, :, CK - 1:CK], Act.Exp, scale=-1.0)
                        nc.vector.tensor_mul(rt_t, projs["r"], expA)
                        nc.vector.tensor_mul(kt_t, projs["k"], expB)
                        for mt in range(CT):
                            for c in range(NCH):
                                csl = slice(c * CK, (c + 1) * CK)
                                nc.vector.tensor_scalar_mul(
                                    kh_t[:, mt, csl], kt_t[:, mt, csl],
                                    wtotW[:, mt, c:c + 1])

                        if _SUB < 4:
                            continue
                        ygT = pw.tile([128, CT, W], bf, tag="ygT")
                        for c in range(NCH):
                            csl = slice(c * CK, (c + 1) * CK)
                            VT = pw.tile([128, CH], bf, tag="VT")
                            KhT = pw.tile([128, CH], bf, tag="KhT")
                            for jt in range(CT):
                                pt = psT.tile([128, 128], bf, tag="tr")
                                nc.tensor.transpose(pt, projs["v"][:, jt, csl],
                                                    ident)
                                nc.scalar.activation(
                                    VT[:, 128 * jt:128 * (jt + 1)], pt,
                                    Act.Identity)
                                pt2 = psT.tile([128, 128], bf, tag="tr")
                                nc.tensor.transpose(pt2, kh_t[:, jt, csl], ident)
                                nc.scalar.activation(
                                    KhT[:, 128 * jt:128 * (jt + 1)], pt2,
                                    Act.Identity)

                            if _SUB < 5:
                                continue
                            # d_i = sum_k r*u*k per head -> dT [tok, 8]
                            ps_d = psB.tile([128, CH], f32, tag="wkvE")
                            for jt in range(CT):
                                nc.tensor.matmul(
                                    ps_d[0:2, 128 * jt:128 * (jt + 1)],
                                    lhsT=blkdiag, rhs=rk_t[:, jt, csl],
                                    start=True, stop=True)
                            d8 = pw.tile([128, CK], bf, tag="d8")
                            for jt in range(CT):
                                nc.vector.tensor_copy(
                                    d8[32 * jt:32 * jt + 2, :],
                                    ps_d[0:2, 128 * jt:128 * (jt + 1)])
                            ptd = psT.tile([128, 128], bf, tag="tr")
                            nc.tensor.transpose(ptd, d8, ident)
                            dT = pw.tile([128, CT, 2], bf, tag="dT")
                            nc.vector.tensor_copy(
                                dT, ptd.rearrange("p (a b) -> p a b",
                                                  b=32)[:, :, 0:2])

                            if _SUB < 6:
                                continue
                            psO = psB.tile([128, CH], f32, tag="wkvO")
                            psE = psB.tile([128, CH], f32, tag="wkvE")
                            psD = psB.tile([128, CH], f32, tag="wkvD")
                            Am = pw.tile([128, CH], bf, tag="Am")
                            Am2 = pw.tile([128, CH], bf, tag="Am2")
                            for h in range(NH):
                                poff = HS * (h % 2)
                                jt = h // 2
                                bank = psE if h % 2 == 0 else psD
                                nc.tensor.matmul(
                                    bank[:, 128 * (h // 2):128 * (h // 2 + 1)],
                                    lhsT=kt_t[poff:poff + HS, jt, csl],
                                    rhs=rt_t[poff:poff + HS, jt, csl],
                                    start=True, stop=True,
                                    tile_position=(poff, 0))
                            for h in range(NH):
                                bank = psE if h % 2 == 0 else psD
                                am = Am if h % 2 == 0 else Am2
                                nc.vector.tensor_mul(
                                    am[:, 128 * (h // 2):128 * (h // 2 + 1)],
                                    bank[:, 128 * (h // 2):128 * (h // 2 + 1)],
                                    maskstr)
                            for h in range(NH):
                                poff = HS * (h % 2)
                                jt = h // 2
                                am = Am if h % 2 == 0 else Am2
                                nc.tensor.matmul(
                                    psO[:, HS * h:HS * (h + 1)],
                                    lhsT=am[:, 128 * (h // 2):128 * (h // 2 + 1)],
                                    rhs=VT[:, HS * h:HS * (h + 1)],
                                    start=True, stop=False)
                                nc.tensor.matmul(
                                    psO[:, HS * h:HS * (h + 1)],
                                    lhsT=rt_t[poff:poff + HS, jt, csl],
                                    rhs=S_cur[poff:poff + HS, jt, :],
                                    start=False, stop=True,
                                    tile_position=(poff, 0))

                            O_sb = pw.tile([128, CH], bf, tag="O_sb")
                            for h in range(NH):
                                nc.vector.scalar_tensor_tensor(
                                    out=O_sb[:, HS * h:HS * (h + 1)],
                                    in0=VT[:, HS * h:HS * (h + 1)],
                                    scalar=dT[:, h // 2, (h % 2):(h % 2) + 1],
                                    in1=psO[:, HS * h:HS * (h + 1)],
                                    op0=Alu.mult, op1=Alu.add)

                            if _SUB < 7:
                                continue
                            psSe = psB.tile([128, CT, HS], f32, tag="wkvSE")
                            psSd = psB.tile([128, CT, HS], f32, tag="wkvSD")
                            for h in range(NH):
                                poff = HS * (h % 2)
                                jt = h // 2
                                bank = psSe if h % 2 == 0 else psSd
                                nc.tensor.matmul(
                                    bank[poff:poff + HS, jt, :],
                                    lhsT=KhT[:, HS * h:HS * (h + 1)],
                                    rhs=VT[:, HS * h:HS * (h + 1)],
                                    start=True, stop=True,
                                    tile_position=(0, poff))
                            S_new = sp.tile([128, CT, HS], bf, tag="S")
                            for jt in range(CT):
                                nc.vector.scalar_tensor_tensor(
                                    out=S_new[0:HS, jt, :],
                                    in0=S_cur[0:HS, jt, :],
                                    scalar=wtotW[0:HS, jt, c:c + 1],
                                    in1=psSe[0:HS, jt, :],
                                    op0=Alu.mult, op1=Alu.add)
                                nc.vector.scalar_tensor_tensor(
                                    out=S_new[HS:128, jt, :],
                                    in0=S_cur[HS:128, jt, :],
                                    scalar=wtotW[HS:128, jt, c:c + 1],
                                    in1=psSd[HS:128, jt, :],
                                    op0=Alu.mult, op1=Alu.add)
                            S_cur = S_new

                            if _SUB < 8:
                                continue
                            # GroupNorm per head
                            stt = pw.tile([128, NH, 6], f32, tag="gn_st")
                            mv = pw.tile([128, NH, 2], f32, tag="gn_mv")
                            for h in range(NH):
                                nc.vector.bn_stats(stt[:, h, :],
                                                   O_sb[:, HS * h:HS * (h + 1)])
                                nc.vector.bn_aggr(mv[:, h, :], stt[:, h, :])
                            sd8 = pw.tile([128, NH], f32, tag="gn_sd")
                            nc.scalar.activation(
                                sd8.rearrange("p (c u) -> p c u", u=1),
                                mv[:, :, 1:2], Act.Sqrt, bias=epst, scale=1.0)
                            rs8 = pw.tile([128, NH], f32, tag="gn_rs")
                            nc.vector.reciprocal(rs8, sd8)
                            nm8 = pw.tile([128, NH], f32, tag="gn_nm")
                            nc.vector.tensor_mul(
                                nm8.rearrange("p (c u) -> p c u", u=1),
                                mv[:, :, 0:1],
                                rs8.rearrange("p (c u) -> p c u", u=1))
                            nc.vector.tensor_scalar_mul(nm8, nm8, -1.0)
                            Ogn = pw.tile([128, CH], bf, tag="Ogn")
                            for h in range(NH):
                                nc.scalar.activation(
                                    Ogn[:, HS * h:HS * (h + 1)],
                                    O_sb[:, HS * h:HS * (h + 1)], Act.Identity,
                                    bias=nm8[:, h:h + 1], scale=rs8[:, h:h + 1])
                            if not skip_gn_affine:
                                t1g = pw.tile([128, CH], bf, tag="gn_t1")
                                nc.vector.tensor_mul(t1g, Ogn, gnw_r)
                                nc.vector.tensor_add(Ogn, t1g, gnb_r)

                            for jt in range(CT):
                                pt3 = psT.tile([128, 128], bf, tag="tr")
                                nc.tensor.transpose(
                                    pt3, Ogn[:, 128 * jt:128 * (jt + 1)], ident)
                                nc.vector.scalar_tensor_tensor(
                                    out=ygT[:, jt, csl], in0=pt3, scalar=ISCW,
                                    in1=projs["g"][:, jt, csl],
                                    op0=Alu.mult, op1=Alu.mult)

                        if _SUB < 10:
                            continue
                        delta = pb.tile([128, DT, W], bf, tag="delta")
                        for mt in range(DT):
                            ps = psA.tile([128, W], f32, tag="mm")
                            mm_chain(ps,
                                     lambda kt, mt=mt:
                                         wo[:, kt, mt * 128:(mt + 1) * 128],
                                     lambda kt: ygT[:, kt, :], CT)
                            nc.scalar.activation(delta[:, mt, :], ps,
                                                 Act.Identity, scale=ISCW)

                        # bf16 payload: halves NeuronLink bytes on the AR
                        bin_ = dram.tile([128, DT, W], bf, tag="bin")
                        bout = dram.tile([128, DT, W], bf, tag="bout")
                        nc.sync.dma_start(out=bin_, in_=delta)
                        nc.gpsimd.collective_compute(
                            "AllReduce", Alu.add, replica_groups=PAIRS,
                            ins=[bin_.opt()], outs=[bout.opt()])
                        if pend is not None:
                            tm_apply(pend)
                        pend = (bout, sl)
                    if pend is not None and _SUB >= 10:
                        tm_apply(pend, last=True)

                # ---------------- channel mix ----------------
                if _STOP < 2 + 2 * l:
                    continue
                with tc.tile_pool(name=f"wcm{l}", bufs=1) as wp2, \
                     tc.tile_pool(name=f"vc2{l}", bufs=1) as vp2, \
                     tc.tile_pool(name=f"cma{l}", bufs=2) as pw2, \
                     tc.tile_pool(name=f"cap{l}", bufs=1) as pa2, \
                     tc.tile_pool(name=f"psc{l}", bufs=4, space="PSUM") as psC:

                    # chunked loads (one DMA per 512-col group) so the first
                    # mt-tile matmuls start before the full 13MB lands
                    cwk = wp2.tile([128, DT, FT * 128], bf, tag="cwk")
                    cwr = wp2.tile([128, DT, D], bf, tag="cwr")
                    cwv = wp2.tile([128, FT, D], bf, tag="cwv")
                    for q in range(3):
                        nc.sync.dma_start(
                            out=cwk[:, :, 512 * q:512 * (q + 1)],
                            in_=wag_out[OFF['cWk'] + q:OFF['cWk'] + 3 * DT:3]
                            .rearrange("j p c -> p j c"))
                    for q in range(2):
                        nc.sync.dma_start(
                            out=cwr[:, :, 512 * q:512 * (q + 1)],
                            in_=wag_out[OFF['cWr'] + q:OFF['cWr'] + 2 * DT:2]
                            .rearrange("j p c -> p j c"))
                        nc.sync.dma_start(
                            out=cwv[:, :, 512 * q:512 * (q + 1)],
                            in_=wag_out[OFF['cWv'] + q:OFF['cWv'] + 2 * FT:2]
                            .rearrange("j p c -> p j c"))
                    if skip_ln_w:
                        ln2w = None
                    else:
                        ln2w = vp2.tile([128, DT], f32, tag="ln2w")
                        nc.sync.dma_start(
                            out=ln2w, in_=vb_d[:, OFFV[('ln2', l)]:
                                              OFFV[('ln2', l)] + DT])
                    ckm = vp2.tile([128, DT], f32, tag="ckm")
                    nc.sync.dma_start(
                        out=ckm, in_=vb_d[:, OFFV[('ck_maa', l)]:
                                          OFFV[('ck_maa', l)] + DT])
                    crm = vp2.tile([128, DT], f32, tag="crm")
                    nc.sync.dma_start(
                        out=crm, in_=vb_d[:, OFFV[('cr_maa', l)]:
                                          OFFV[('cr_maa', l)] + DT])

                    def cm_apply(pend, last=False):
                        bout_p, sl_p, rr_p = pend
                        eng = nc.gpsimd if last else nc.vector
                        ar2 = pa2.tile([128, DT, W], bf, tag="ar2")
                        nc.sync.dma_start(out=ar2, in_=bout_p)
                        xwf2 = pa2.tile([128, DT, W], f32, tag="xwf2")
                        nc.sync.dma_start(
                            out=xwf2, in_=xres_t[:, :, sl_p]
                            .rearrange("j p w -> p j w"))
                        tmp2 = pa2.tile([128, DT, W], f32, tag="tmp2")
                        eng.tensor_mul(tmp2, rr_p, ar2)
                        eng.tensor_add(xwf2, xwf2, tmp2)
                        xnb2 = pa2.tile([128, DT, W], bf, tag="xnb2")
                        if last:
                            nc.gpsimd.tensor_copy(xnb2, xwf2)
                        else:
                            nc.scalar.activation(xnb2, xwf2, Act.Identity)
                        nc.sync.dma_start(
                            out=xres_t[:, :, sl_p].rearrange("j p w -> p j w"),
                            in_=xwf2)
                        nc.sync.dma_start(
                            out=xresb_t[:, :, sl_p].rearrange("j p w -> p j w"),
                            in_=xnb2)

                    pend = None
                    xln2_prev = None
                    for w in range(NW):
                        sl = slice(w * W, (w + 1) * W)
                        xwb2 = pw2.tile([128, DT, W], bf, tag="xwb2")
                        nc.sync.dma_start(
                            out=xwb2, in_=xresb_t[:, :, sl]
                            .rearrange("j p w -> p j w"))
                        xln2 = layer_norm(pw2, psC, ln2w, xwb2)
                        sx2 = token_shift(pw2, xln2, xln2_prev, w)
                        kx = pw2.tile([128, DT, W], bf, tag="kx")
                        rx = pw2.tile([128, DT, W], bf, tag="rx")
                        for j in range(DT):
                            nc.vector.scalar_tensor_tensor(
                                out=kx[:, j, :], in0=sx2[:, j, :],
                                scalar=ckm[:, j:j + 1], in1=xln2[:, j, :],
                                op0=Alu.mult, op1=Alu.add)
                            nc.vector.scalar_tensor_tensor(
                                out=rx[:, j, :], in0=sx2[:, j, :],
                                scalar=crm[:, j:j + 1], in1=xln2[:, j, :],
                                op0=Alu.mult, op1=Alu.add)

                        kk = pw2.tile([128, FT, W], bf, tag="kk")
                        for mt in range(FT):
                            ps = psC.tile([128, W], f32, tag="mm")
                            mm_chain(ps,
                                     lambda kt, mt=mt:
                                         cwk[:, kt, mt * 128:(mt + 1) * 128],
                                     lambda kt: kx[:, kt, :], DT)
                            kk0 = pw2.tile([128, W], bf, tag="kk0")
                            nc.scalar.activation(kk0, ps, Act.Relu, scale=ISCW)
                            nc.vector.tensor_mul(kk[:, mt, :], kk0, kk0)

                        delta2 = pw2.tile([128, DT, W], bf, tag="delta2")
                        for mt in range(DT):
                            ps = psC.tile([128, W], f32, tag="mm")
                            mm_chain(ps,
                                     lambda kt, mt=mt:
                                         cwv[:, kt, mt * 128:(mt + 1) * 128],
                                     lambda kt: kk[:, kt, :], FT)
                            nc.scalar.activation(delta2[:, mt, :], ps,
                                                 Act.Identity, scale=ISCW)

                        rr = pw2.tile([128, DT, W], bf, tag="rr")
                        for mt in range(DT):
                            ps = psC.tile([128, W], f32, tag="mm")
                            mm_chain(ps,
                                     lambda kt, mt=mt:
                                         cwr[:, kt, mt * 128:(mt + 1) * 128],
                                     lambda kt: rx[:, kt, :], DT)
                            nc.scalar.activation(rr[:, mt, :], ps, Act.Sigmoid,
                                                 scale=ISCW)

                        bin2 = dram.tile([128, DT, W], bf, tag="bin")
                        bout2 = dram.tile([128, DT, W], bf, tag="bout")
                        nc.sync.dma_start(out=bin2, in_=delta2)
                        nc.gpsimd.collective_compute(
                            "AllReduce", Alu.add, replica_groups=PAIRS,
                            ins=[bin2.opt()], outs=[bout2.opt()])
                        if pend is not None:
                            cm_apply(pend)
                        pend = (bout2, sl, rr)
                        xln2_prev = xln2
                    if pend is not None:
                        cm_apply(pend, last=True)

            # ---------------- final LN ----------------
            with tc.tile_pool(name="fin", bufs=2) as pf, \
                 tc.tile_pool(name="psf", bufs=2, space="PSUM") as psF, \
                 tc.tile_pool(name="vecf", bufs=1) as vf:
                if skip_ln_w:
                    lnfw = None
                else:
                    lnfw = vf.tile([128, DT], f32, tag="lnfw")
                    nc.sync.dma_start(
                        out=lnfw,
                        in_=vb_d[:, OFFV['lnf']:OFFV['lnf'] + DT])
                for w in range(NWC):
                    sl = slice(w * WC, (w + 1) * WC)
                    xwb3 = pf.tile([128, DT, WC], bf, tag="xwb3")
                    nc.sync.dma_start(out=xwb3, in_=xsrc_bf(2 * L, sl))
                    yw = layer_norm(pf, psF, lnfw, xwb3)
                    for jl in range(DT):
                        nc.sync.dma_start(out=y_d[jl, :, sl],
                                          in_=yw[:, jl, :])
    if split_waits:
        _split_sync_waits(nc, scratch=wsplit_sem)
    return nc


# ===================== host side =====================

_CACHE = {}
_RUNNER_CACHE = {}


class _Runner:
    """AOT-compiled PJRT executor for a built Bass module.

    Mirrors bass2jax.run_bass_via_pjrt's lowering exactly, but (a) traces,
    lowers and XLA-compiles ONCE, and (b) takes device-resident global
    arrays, so a steady-state call is just dispatch + NEFF execution —
    no re-trace, no re-compile, no host->device weight re-upload.
    """

    def __init__(self, nc, n_cores=8):
        import jax
        import jax.numpy as jnp
        from jax.sharding import Mesh, NamedSharding, PartitionSpec
        from jax.experimental.shard_map import shard_map
        from concourse import bass2jax

        bass2jax.install_neuronx_cc_hook()
        self.nc = nc
        self.n_cores = n_cores
        self._jax = jax

        partition_name = (nc.partition_id_tensor.name
                          if nc.partition_id_tensor else None)
        in_names, out_names, out_avals, zero_specs = [], [], [], []
        for alloc in nc.m.functions[0].allocations:
            if not isinstance(alloc, mybir.MemoryLocationSet):
                continue
            name = alloc.memorylocations[0].name
            if alloc.kind == "ExternalInput":
                if name != partition_name:
                    in_names.append(name)
            elif alloc.kind == "ExternalOutput":
                assert alloc.tensor_shape is not None
                shape = tuple(alloc.tensor_shape)
                dtype = mybir.dt.np(alloc.dtype)
                out_names.append(name)
                out_avals.append(jax.core.ShapedArray(shape, dtype))
                zero_specs.append((shape, dtype))

        self.dbg_name = None
        if nc.dbg_addr is not None:
            if nc.dbg_callbacks:
                raise RuntimeError("dbg_callbacks unsupported under axon")
            self.dbg_name = nc.dbg_addr.name

        n_params = len(in_names)
        self.param_names = list(in_names)
        self.out_names = list(out_names)
        self.out_avals = out_avals
        in_names_full = list(in_names) + list(out_names)
        if partition_name is not None:
            in_names_full.append(partition_name)
        donate = tuple(range(n_params, n_params + len(out_names)))

        def _body(*args):
            operands = list(args)
            if partition_name is not None:
                operands.append(bass2jax.partition_id_tensor())
            outs = bass2jax._bass_exec_p.bind(
                *operands,
                out_avals=tuple(out_avals),
                in_names=tuple(in_names_full),
                out_names=tuple(out_names),
                lowering_input_output_aliases=(),
                sim_require_finite=True,
                sim_require_nnan=True,
                nc=nc,
            )
            return tuple(outs)

        devices = jax.devices()[:n_cores]
        assert len(devices) == n_cores
        mesh = Mesh(np.asarray(devices), ("core",))
        self.sharding = NamedSharding(mesh, PartitionSpec("core"))
        in_specs = (PartitionSpec("core"),) * (n_params + len(out_names))
        out_specs = (PartitionSpec("core"),) * len(out_names)

        # global (concatenated) avals for AOT lowering
        in_sds = []
        for nm in in_names:
            shape, dtype = self._global_spec(nm)
            in_sds.append(jax.ShapeDtypeStruct(shape, dtype))
        zero_sds = [
            jax.ShapeDtypeStruct((n_cores * s[0],) + tuple(s[1:]), d)
            for (s, d) in zero_specs
        ]

        def _compile():
            jitted = jax.jit(
                shard_map(_body, mesh=mesh, in_specs=in_specs,
                          out_specs=out_specs, check_rep=False),
                donate_argnums=donate, keep_unused=True)
            return jitted.lower(*in_sds, *zero_sds).compile()

        try:
            self.compiled = bass2jax.fast_dispatch_compile(_compile)
        except Exception:
            self.compiled = _compile()

        self._make_zeros = jax.jit(
            lambda: tuple(jnp.zeros(s.shape, s.dtype) for s in zero_sds),
            out_shardings=tuple(self.sharding for _ in zero_sds))

    def _global_spec(self, name):
        """global concat shape/dtype for an ExternalInput name"""
        for alloc in self.nc.m.functions[0].allocations:
            if (isinstance(alloc, mybir.MemoryLocationSet)
                    and alloc.memorylocations[0].name == name):
                shape = tuple(alloc.tensor_shape)
                if name == self.dbg_name:
                    # uint32[1,2] view of the 8-byte dbg PA (x64 off)
                    return (self.n_cores, 2), np.uint32
                return ((self.n_cores * shape[0],) + shape[1:],
                        mybir.dt.np(alloc.dtype))
        raise KeyError(name)

    def put(self, conc):
        """device_put global arrays (dict name->np) -> resident arg list"""
        jax = self._jax
        args = []
        for nm in self.param_names:
            if nm == self.dbg_name and nm not in conc:
                a = np.zeros((self.n_cores, 2), np.uint32)
            else:
                a = conc[nm]
            args.append(jax.device_put(a, self.sharding))
        return args

    def run(self, dev_args):
        """execute; returns tuple of global sharded output arrays"""
        return self.compiled(*dev_args, *self._make_zeros())


def _get_runner(T, skip_gn_affine, skip_ln_w):
    key = (T, skip_gn_affine, skip_ln_w)
    if key not in _RUNNER_CACHE:
        _RUNNER_CACHE[key] = _Runner(_get_nc(T, skip_gn_affine, skip_ln_w))
    return _RUNNER_CACHE[key]


def _concat_inputs(inputs, T):
    in_maps = [_prep_core_inputs(inputs, c, T) for c in range(8)]
    return {n: np.concatenate([m[n] for m in in_maps], axis=0)
            for n in in_maps[0]}


def _prep_core_inputs(inputs, core, T):
    b, half = core // 2, core % 2
    hsl = slice(half * CH, (half + 1) * CH)
    fsl = slice(half * FT * 128, (half + 1) * FT * 128)

    def kt_tiles(w):
        return np.ascontiguousarray(
            w.reshape(w.shape[0] // 128, 128, w.shape[1]).astype(BF16))

    def vec_tiles(v):
        return np.ascontiguousarray(v.reshape(-1, 128).T.astype(np.float32))

    def bricks(w):
        """[n,128,C] f32-ish, C % 512 == 0 -> [n*C/512, 128, 512] fp8*SCW"""
        n, p, cc = w.shape
        assert p == 128 and cc % 512 == 0
        return np.ascontiguousarray(
            (w.astype(np.float32) * SCW)
            .reshape(n, 128, cc // 512, 512).transpose(0, 2, 1, 3)
            .reshape(-1, 128, 512).astype(BF16))

    # x ships full per core (device-resident across calls)
    xt = inputs["x"][b][:T].T.astype(BF16)        # [D, T]
    xt = xt.reshape(DT, 128, T)
    out = {"x": np.ascontiguousarray(xt)}

    # big-weight blob for this core's half, 4-way sharded by batch index
    blob = np.zeros((NBR, 128, 512), BF16)
    for l in range(L):
        OFF = _woff(l)
        blob[OFF['Wr']:OFF['Wr'] + 8] = bricks(kt_tiles(inputs["Wr"][l][:, hsl]))
        blob[OFF['Wk']:OFF['Wk'] + 8] = bricks(kt_tiles(inputs["Wk"][l][:, hsl]))
        blob[OFF['Wv']:OFF['Wv'] + 8] = bricks(kt_tiles(inputs["Wv"][l][:, hsl]))
        blob[OFF['Wg']:OFF['Wg'] + 8] = bricks(kt_tiles(inputs["Wg"][l][:, hsl]))
        blob[OFF['Wo']:OFF['Wo'] + 8] = bricks(kt_tiles(inputs["Wo"][l][hsl, :]))
        blob[OFF['cWk']:OFF['cWk'] + 24] = bricks(kt_tiles(inputs["cWk"][l][:, fsl]))
        blob[OFF['cWv']:OFF['cWv'] + 24] = bricks(kt_tiles(inputs["cWv"][l][fsl, :]))
        blob[OFF['cWr']:OFF['cWr'] + 16] = bricks(kt_tiles(inputs["cWr"][l]))
        # small weights, packed into spare bricks
        tm1 = kt_tiles(inputs["tm_w1"][l])          # [8, 128, 160]
        for t in range(DT):
            blob[OFF['tm_w1'] + t // 3, :, 160 * (t % 3):160 * (t % 3) + 160] \
                = tm1[t]
        tw2 = np.zeros((2, 128, D), np.float32)
        for f in range(5):
            tw2[f // 4, E * (f % 4):E * (f % 4) + E] = inputs["tm_w2"][l, f]
        for j in range(2):
            for q in range(2):
                blob[OFF['tm_w2'] + 2 * j + q] = \
                    tw2[j, :, 512 * q:512 * (q + 1)].astype(BF16)
        td1 = kt_tiles(inputs["td_w1"][l])          # [8, 128, 64]
        for t in range(DT):
            blob[OFF['td_w1'], :, 64 * t:64 * (t + 1)] = td1[t]
        blob[OFF['td_w2'], 0:ED, :] = inputs["td_w2"][l][:, hsl].astype(BF16)
    # extra brick: ident + blkdiag constants
    extra = np.zeros((1, 128, 512), BF16)
    extra[0, :, 0:128] = np.eye(128, dtype=BF16)
    extra[0, 0:64, 128] = 1
    extra[0, 64:128, 129] = 1
    out["wblob"] = np.ascontiguousarray(
        np.concatenate([blob, extra], axis=0))

    skip_gn, skip_ln = _flags(inputs)
    OFFV, VC = _vlayout(skip_gn, skip_ln)
    vb = np.zeros((128, VC), np.float32)
    vb[:, OFFV['maskstr']:OFFV['maskstr'] + 128] = \
        np.triu(np.ones((128, 128), np.float32), 1)
    for l in range(L):
        for n in ["x_maa", "w_maa", "k_maa", "v_maa", "r_maa", "g_maa",
                  "ck_maa", "cr_maa"]:
            vb[:, OFFV[(n, l)]:OFFV[(n, l)] + DT] = vec_tiles(inputs[n][l])
        vb[:, OFFV[('tdb', l)]:OFFV[('tdb', l)] + CT] = \
            vec_tiles(inputs["time_decay"][l].reshape(-1)[hsl])
        vb[:, OFFV[('u', l)]:OFFV[('u', l)] + CT] = \
            vec_tiles(inputs["time_first"][l].reshape(-1)[hsl])
        if not skip_ln:
            vb[:, OFFV[('ln1', l)]:OFFV[('ln1', l)] + DT] = \
                vec_tiles(inputs["ln1_w"][l])
            vb[:, OFFV[('ln2', l)]:OFFV[('ln2', l)] + DT] = \
                vec_tiles(inputs["ln2_w"][l])
        if not skip_gn:
            vb[:, OFFV[('gnw', l)]:OFFV[('gnw', l)] + CH] = \
                inputs["gn_w"][l][hsl][None, :]
            vb[:, OFFV[('gnb', l)]:OFFV[('gnb', l)] + CH] = \
                inputs["gn_b"][l][hsl][None, :]
    if not skip_ln:
        vb[:, OFFV['lnf']:OFFV['lnf'] + DT] = vec_tiles(inputs["ln_out_w"])
    out["vblob"] = vb
    return out


def _get_nc(T, skip_gn_affine, skip_ln_w):
    key = (T, skip_gn_affine, skip_ln_w)
    if key not in _CACHE:
        nc = bass.Bass(trn_type="TRN2", num_devices=8)
        build(nc, T, skip_gn_affine, skip_ln_w)
        # the module is immutable after build; memoize its (large) JSON
        # serialization so repeated run_bass_kernel_spmd calls don't redo it
        raw = nc.to_json_bytes()
        nc.to_json_bytes = lambda: raw
        _CACHE[key] = nc
    return _CACHE[key]


def _flags(inputs):
    skip_gn = bool(np.all(inputs["gn_w"] == 1.0)
                   and np.all(inputs["gn_b"] == 0.0))
    skip_ln = bool(np.all(inputs["ln1_w"] == 1.0)
                   and np.all(inputs["ln2_w"] == 1.0)
                   and np.all(inputs["ln_out_w"] == 1.0))
    return skip_gn, skip_ln


def _gather_output(y_global, T):
    """y_global: [8*DT, 128, T] -> full [B, T, D] f32.
    Both cores of a pair compute the full (identical) y; use the even one."""
    y = np.asarray(y_global).reshape(8, DT, 128, T)
    outs = [y[2 * b].transpose(2, 0, 1).reshape(T, D) for b in range(B)]
    return np.stack(outs).astype(np.float32)


def kernel(**inputs):
    inputs = {k: np.asarray(v) for k, v in inputs.items()}
    T = inputs["x"].shape[1]
    skip_gn, skip_ln = _flags(inputs)
    runner = _get_runner(T, skip_gn, skip_ln)
    conc = _concat_inputs(inputs, T)
    dev = runner.put(conc)
    outs = runner.run(dev)
    return _gather_output(outs[0], T)


if __name__ == "__main__":
    import sys
    Tk = int(sys.argv[1]) if len(sys.argv) > 1 else 2048
    z = np.load('/tmp/inputs.npz')
    inputs = {k: z[k] for k in z.files}
    inputs["x"] = np.ascontiguousarray(inputs["x"][:, :Tk])
    act = kernel(**inputs)
    import np_ref
    exp = np_ref.np_reference(inputs)
    rel = np.linalg.norm(act - exp) / np.linalg.norm(exp)
    print("rel l2 vs np reference:", rel)
    print("max abs diff:", np.abs(act - exp).max())



# revision 73
# speedup vs baseline: 1.0042x; 1.0042x over previous
"""RWKV6 (nn_ExtendedMemory) Trainium2 kernel — 8 NeuronCores, v3.

Sharding: core c -> batch c//2, tensor-parallel half c%2 (8 of 16 heads,
half of the FFN columns). Two pairwise bf16 AllReduces per layer window
(Wo output and cWv output) via collective_compute.

v3 host/runtime design (supersedes v2's tunnel-upload optimizations):
  - _Runner AOT-compiles the module ONCE (jit(shard_map).lower().compile()
    via bass2jax's fast-dispatch path) and keeps all inputs device-resident
    across calls; a steady-state call is pure dispatch + NEFF execution.
  - Inputs are consolidated into 4 operands (partition_id, x, wblob, vblob)
    because the axon PJRT runtime costs ~40us per operand per call.
    wblob: all matmul weights as [229,128,512] bf16 bricks (+ident/blkdiag
    constants); vblob: all f32 vectors/constants packed as [128, VC].
  - No input AllGathers / output ReduceScatter: each core receives its full
    half-set of weights and full x, and writes the full (pair-identical) y.
  - LN/elementwise chain runs bf16 (PE matmuls bf16); the residual stream
    stays f32 in DRAM with a bf16 mirror for LN input. Batched 3D vector
    ops (token shift, LN normalize via stride-0 broadcast APs, residual
    applies) and ACT-engine offload (PSUM evacuation, relu, delta casts)
    keep DVE instruction count down.
"""

import os
import numpy as np
import ml_dtypes

import concourse.bass as bass
import concourse.mybir as mybir
import concourse.tile as tile
from concourse.bass_utils import run_bass_kernel_spmd

dt = mybir.dt
Alu = mybir.AluOpType
Act = mybir.ActivationFunctionType
BF16 = ml_dtypes.bfloat16
F8 = ml_dtypes.float8_e4m3
SCW = 1.0             # weight pre-scale (1.0 = bf16 blob, no scaling)
ISCW = 1.0 / SCW

L, D, HS, E, ED, FE = 2, 1024, 64, 32, 64, 3
H = D // HS            # 16 heads total
B = 4
EPS = 1e-5
NH = H // 2            # 8 heads per core
CH = NH * HS           # 512 channels per core
DT = D // 128          # 8 D-tiles
CT = CH // 128         # 4 chan-tiles per core
FT = FE * D // 2 // 128  # 12 ffn-tiles per core
CK = 128               # wkv chunk
WIN = 256              # token window

PAIRS = [[0, 1], [2, 3], [4, 5], [6, 7]]
if os.environ.get("KSELF"):      # timing probe: no inter-core sync
    PAIRS = [[c] for c in range(8)]
COLS = [[0, 2, 4, 6], [1, 3, 5, 7]]

# big-weight brick blob: [nbricks, 128, 512] bf16 per half-set
BPL = 114              # bricks per layer (104 big + 9 small + 1 pad)
NBR = BPL * L          # 228 bricks per half-set
SHARD = NBR // 4       # 57 bricks shipped per core


def _woff(l):
    o = BPL * l
    return dict(Wr=o, Wk=o + 8, Wv=o + 16, Wg=o + 24, Wo=o + 32,
                cWk=o + 40, cWv=o + 64, cWr=o + 88,
                tm_w1=o + 104, tm_w2=o + 107, td_w1=o + 111, td_w2=o + 112)


IDBRICK = NBR          # extra brick: ident [*,0:128], blkdiag [*,128:130]


def _vlayout(skip_gn_affine, skip_ln_w):
    """column layout of the packed f32 vector blob [128, VC]"""
    off = {}
    c = 0
    off['maskstr'] = c
    c += 128
    for l in range(L):
        for n in ["x_maa", "w_maa", "k_maa", "v_maa", "r_maa", "g_maa",
                  "ck_maa", "cr_maa"]:
            off[(n, l)] = c
            c += DT
        off[('tdb', l)] = c
        c += CT
        off[('u', l)] = c
        c += CT
        if not skip_ln_w:
            off[('ln1', l)] = c
            c += DT
            off[('ln2', l)] = c
            c += DT
        if not skip_gn_affine:
            off[('gnw', l)] = c
            c += CH
            off[('gnb', l)] = c
            c += CH
    if not skip_ln_w:
        off['lnf'] = c
        c += DT
    return off, c


TC = tile.TileContext


_wsplit_counter = [0]


def _split_sync_waits(nc, scratch=None, max_waits=1):
    """walrus in this container rejects >1 sync wait per instruction.

    For single-queue engines (PE/DVE/ACT/SP) excess waits move onto
    same-engine standalone EventSemaphore instructions placed immediately
    before the owner (engine streams are strict FIFO, so this is
    equivalent). GpSimd fans instructions across 8 Q7 queues, so a
    standalone wait there guards nothing — instead its waits are relayed:
    SP waits each semaphore (EVSEM chain), then bumps a scratch semaphore
    that the Pool instruction waits on (its single allowed wait)."""
    if scratch is None:
        scratch = nc.alloc_semaphore("wsplit_scratch")
    scratch_count = [0]

    def evsem(engine, waits, updates=()):
        _wsplit_counter[0] += 1
        ev = mybir.InstEventSemaphore(
            name=f"I-wsplit-{_wsplit_counter[0]}", ins=[], outs=[])
        ev.engine = engine
        ev.sync_info = mybir.SyncInfo(on_wait=list(waits),
                                      on_update=list(updates))
        return ev

    sp = mybir.EngineType.Activation
    for f in nc.m.functions:
        for bb in f.blocks:
            out = []
            changed = False
            for inst in bb.instructions:
                si = inst.sync_info
                if si is not None and len(si.on_wait) > max_waits:
                    waits = list(si.on_wait)
                    changed = True
                    if inst.engine == mybir.EngineType.Pool:
                        for wv in waits:
                            out.append(evsem(sp, [wv]))
                        scratch_count[0] += 1
                        out.append(evsem(sp, [], [mybir.SyncUpdate(
                            sync_type="semaphore", id=scratch.num,
                            update_mode="sem-inc", update_value=1)]))
                        keep = [mybir.SyncWait(
                            sync_type="semaphore", id=scratch.num,
                            wait_mode="sem-ge-imm",
                            wait_value=scratch_count[0])]
                    else:
                        extra, keep = waits[:-max_waits], waits[-max_waits:]
                        while extra:
                            chunk, extra = (extra[:max_waits],
                                            extra[max_waits:])
                            out.append(evsem(inst.engine, chunk))
                    inst.sync_info = mybir.SyncInfo(
                        on_wait=keep, on_update=list(si.on_update))
                out.append(inst)
            if changed:
                bb.instructions = out


def build(nc, T, skip_gn_affine, skip_ln_w, split_waits=True):
    W = min(WIN, T)
    assert T % W == 0
    NW = T // W
    NCH = W // CK or 1
    assert W % CK == 0
    WC = min(2 * WIN, T)   # wider windows for the final LN
    assert T % WC == 0
    NWC = T // WC

    f32, bf = dt.float32, dt.bfloat16

    def din(name, shape, d=f32):
        return nc.dram_tensor(name, shape, d, kind="ExternalInput")

    x_d = din("x", [DT, 128, T], bf)
    wb_d = din("wblob", [NBR + 1, 128, 512], bf)
    y_d = nc.dram_tensor("y", [DT, 128, T], bf, kind="ExternalOutput")

    OFFV, VC = _vlayout(skip_gn_affine, skip_ln_w)
    vb_d = din("vblob", [128, VC])

    # reserved before the TileContext so Tile's allocator can't recycle it
    wsplit_sem = nc.alloc_semaphore("wsplit_scratch")
    nc.sync.sem_clear(wsplit_sem)

    with TC(nc) as tc:
        import contextlib
        ctx = contextlib.ExitStack()
        with ctx:
            const = ctx.enter_context(tc.tile_pool(name="const", bufs=1))
            dram = ctx.enter_context(tc.tile_pool(name="dramb", bufs=3, space="DRAM"))
            xrp = ctx.enter_context(tc.tile_pool(name="xrp", bufs=1, space="DRAM"))
            xres_t = xrp.tile([DT, 128, T], f32, tag="xres")
            xresb_t = xrp.tile([DT, 128, T], bf, tag="xresb")

            # weights arrive full per core (device-resident across calls),
            # x arrives full per core: no input AllGathers needed.
            wag_out = wb_d

            def xsrc_bf(phase, sl):
                """bf16 LN-input window [128, DT, W] source for a phase."""
                t = x_d if phase == 0 else xresb_t
                return t[:, :, sl].rearrange("j p w -> p j w")

            ident = const.tile([128, 128], bf)
            nc.sync.dma_start(out=ident, in_=wb_d[IDBRICK, :, 0:128])
            maskstr = const.tile([128, 128], f32)   # keep j < i over [j, i]
            nc.sync.dma_start(
                out=maskstr,
                in_=vb_d[:, OFFV['maskstr']:OFFV['maskstr'] + 128])
            blkdiag = const.tile([128, 2], bf)      # col a = partitions 64a..
            nc.sync.dma_start(out=blkdiag, in_=wb_d[IDBRICK, :, 128:130])
            ones_bf = const.tile([128, 1], bf)
            nc.vector.memset(ones_bf, 1.0)
            ones_f = const.tile([128, 1], f32)
            nc.vector.memset(ones_f, 1.0)
            ones_rowb = const.tile([1, 128], bf)
            nc.vector.memset(ones_rowb, 1.0)
            zerosCK = const.tile([128, CK], f32)
            nc.vector.memset(zerosCK, 0.0)
            epst = const.tile([128, 1], f32)
            nc.vector.memset(epst, EPS)

            def jbc(t):
                """broadcast a [128, W] tile over the DT axis (stride-0 AP)"""
                a = t[:, :]
                return bass.AP(tensor=a.tensor, offset=a.offset,
                               ap=[list(a.ap[0]), [0, DT], list(a.ap[1])])

            def layer_norm(pool, ps_pool, lnw_t, xsrc, W=None):
                """LN over channels. xsrc: [128, DT, W] SBUF bf16 window.
                Returns xln [128, DT, W] bf16."""
                if W is None:
                    W = WIN if T >= WIN else T
                W = xsrc.shape[2]
                ps = ps_pool.tile([128, W], f32, tag="mm")
                ps_sq = ps_pool.tile([128, W], f32, tag="mm")
                sqa = pool.tile([128, DT, W], bf, tag="ln_sqa")
                nc.scalar.activation(sqa, xsrc, Act.Square)
                for j in range(DT):
                    nc.tensor.matmul(ps[0:1, :], lhsT=ones_bf,
                                     rhs=xsrc[:, j, :],
                                     start=(j == 0), stop=(j == DT - 1))
                    nc.tensor.matmul(ps_sq[0:1, :], lhsT=ones_bf,
                                     rhs=sqa[:, j, :],
                                     start=(j == 0), stop=(j == DT - 1))
                mu = pool.tile([1, W], f32, tag="ln_mu")
                nc.vector.tensor_scalar_mul(mu, ps[0:1, :], 1.0 / D)
                mub = pool.tile([1, W], bf, tag="ln_mub")
                nc.vector.tensor_copy(mub, mu)
                musq = pool.tile([1, W], f32, tag="ln_musq")
                nc.vector.tensor_mul(musq, mu, mu)
                var = pool.tile([1, W], f32, tag="ln_var")
                nc.vector.scalar_tensor_tensor(out=var, in0=ps_sq[0:1, :],
                                               scalar=1.0 / D, in1=musq,
                                               op0=Alu.mult, op1=Alu.subtract)
                sd = pool.tile([1, W], f32, tag="ln_sd")
                nc.scalar.activation(sd, var, Act.Sqrt, bias=epst[0:1], scale=1.0)
                rstdb = pool.tile([1, W], bf, tag="ln_rstdb")
                with nc.allow_low_precision(reason="bf16 rstd is plenty for LN"):
                    nc.vector.reciprocal(rstdb, sd)
                ps_b = ps_pool.tile([128, W], f32, tag="mm")
                nc.tensor.matmul(ps_b, lhsT=ones_rowb, rhs=mub, start=True,
                                 stop=True)
                mur = pool.tile([128, W], bf, tag="ln_mur")
                nc.vector.tensor_copy(mur, ps_b)
                ps_b2 = ps_pool.tile([128, W], f32, tag="mm")
                nc.tensor.matmul(ps_b2, lhsT=ones_rowb, rhs=rstdb, start=True,
                                 stop=True)
                rstdr = pool.tile([128, W], bf, tag="ln_rstdr")
                nc.vector.tensor_copy(rstdr, ps_b2)
                xln = pool.tile([128, DT, W], bf, tag="ln_out")
                if skip_ln_w:
                    nc.vector.tensor_sub(xln, xsrc, jbc(mur))
                    nc.vector.tensor_mul(xln, xln, jbc(rstdr))
                else:
                    tmp = pool.tile([128, W], bf, tag="ln_tmp")
                    for j in range(DT):
                        nc.vector.tensor_sub(tmp, xsrc[:, j, :], mur)
                        nc.vector.scalar_tensor_tensor(
                            out=xln[:, j, :], in0=tmp, scalar=lnw_t[:, j:j + 1],
                            in1=rstdr, op0=Alu.mult, op1=Alu.mult)
                return xln

            def token_shift(pool, xln, xln_prev, w):
                sx = pool.tile([128, DT, W], bf, tag="sx")
                nc.vector.tensor_sub(sx[:, :, 1:W], xln[:, :, 0:W - 1],
                                     xln[:, :, 1:W])
                if w == 0:
                    nc.vector.tensor_scalar_mul(sx[:, :, 0:1],
                                                xln[:, :, 0:1], -1.0)
                else:
                    nc.vector.tensor_sub(sx[:, :, 0:1],
                                         xln_prev[:, :, W - 1:W],
                                         xln[:, :, 0:1])
                return sx

            def mm_chain(ps, lhsT_f, rhs_f, nkt):
                for kt in range(nkt):
                    nc.tensor.matmul(ps, lhsT=lhsT_f(kt), rhs=rhs_f(kt),
                                     start=(kt == 0), stop=(kt == nkt - 1))

            # ================= layers =================
            _STOP = int(os.environ.get("KSTOP", "99"))
            _SUB = int(os.environ.get("KSUB", "99"))
            for l in range(L):
                if _STOP < 1 + 2 * l:
                    break
                phase_tm = 2 * l      # residual source phase id
                phase_cm = 2 * l + 1
                OFF = _woff(l)

                # ---------------- time mix ----------------
                with tc.tile_pool(name=f"wtm{l}", bufs=1) as wp, \
                     tc.tile_pool(name=f"vec{l}", bufs=1) as vp, \
                     tc.tile_pool(name=f"tma{l}", bufs=2) as pw, \
                     tc.tile_pool(name=f"tmb{l}", bufs=1) as pk, \
                     tc.tile_pool(name=f"tmc{l}", bufs=1) as pb, \
                     tc.tile_pool(name=f"tap{l}", bufs=1) as pa1, \
                     tc.tile_pool(name=f"st{l}", bufs=2) as sp, \
                     tc.tile_pool(name=f"psa{l}", bufs=2, space="PSUM") as psA, \
                     tc.tile_pool(name=f"psb{l}", bufs=1, space="PSUM") as psB, \
                     tc.tile_pool(name=f"pst{l}", bufs=1, space="PSUM") as psT:

                    w_r = wp.tile([128, DT, CH], bf, tag="w_r")
                    w_k = wp.tile([128, DT, CH], bf, tag="w_k")
                    w_v = wp.tile([128, DT, CH], bf, tag="w_v")
                    w_g = wp.tile([128, DT, CH], bf, tag="w_g")
                    for wt, nm in [(w_r, 'Wr'), (w_k, 'Wk'),
                                   (w_v, 'Wv'), (w_g, 'Wg')]:
                        nc.sync.dma_start(
                            out=wt,
                            in_=wag_out[OFF[nm]:OFF[nm] + DT]
                            .rearrange("n p c -> p n c"))
                    tmw1 = wp.tile([128, DT, 5 * E], bf, tag="tmw1")
                    tdw1 = wp.tile([128, DT, ED], bf, tag="tdw1")
                    for j in range(DT):
                        c0 = 160 * (j % 3)
                        nc.sync.dma_start(
                            out=tmw1[:, j, :],
                            in_=wag_out[OFF['tm_w1'] + j // 3, :, c0:c0 + 160])
                        nc.sync.dma_start(
                            out=tdw1[:, j, :],
                            in_=wag_out[OFF['td_w1'], :, 64 * j:64 * (j + 1)])
                    tmw2 = wp.tile([128, 2, D], bf, tag="tmw2")
                    for j in range(2):
                        for q in range(2):
                            nc.sync.dma_start(
                                out=tmw2[:, j, 512 * q:512 * (q + 1)],
                                in_=wag_out[OFF['tm_w2'] + 2 * j + q])
                    tdw2 = wp.tile([ED, CH], bf, tag="tdw2")
                    nc.sync.dma_start(out=tdw2, in_=wag_out[OFF['td_w2'], 0:ED, :])
                    wo = wp.tile([128, CT, D], bf, tag="wo")
                    nc.sync.dma_start(
                        out=wo.rearrange("p j (q c) -> p j q c", c=512),
                        in_=wag_out[OFF['Wo']:OFF['Wo'] + 2 * CT]
                        .rearrange("(j q) p c -> p j q c", q=2))

                    if skip_ln_w:
                        ln1w = None
                    else:
                        ln1w = vp.tile([128, DT], f32, tag="ln1w")
                        nc.sync.dma_start(
                            out=ln1w, in_=vb_d[:, OFFV[('ln1', l)]:
                                              OFFV[('ln1', l)] + DT])
                    maat = {}
                    for n in ["x_maa", "w_maa", "k_maa", "v_maa", "r_maa",
                              "g_maa"]:
                        maat[n] = vp.tile([128, DT], f32, tag=n, name=n)
                        nc.sync.dma_start(
                            out=maat[n],
                            in_=vb_d[:, OFFV[(n, l)]:OFFV[(n, l)] + DT])
                    tdb = vp.tile([128, CT], f32, tag="tdb")
                    nc.sync.dma_start(
                        out=tdb,
                        in_=vb_d[:, OFFV[('tdb', l)]:OFFV[('tdb', l)] + CT])
                    ut = vp.tile([128, CT], f32, tag="ut")
                    nc.sync.dma_start(
                        out=ut, in_=vb_d[:, OFFV[('u', l)]:OFFV[('u', l)] + CT])
                    if not skip_gn_affine:
                        gnw_r = vp.tile([128, CH], bf, tag="gnw_r")
                        nc.sync.dma_start(
                            out=gnw_r, in_=vb_d[:, OFFV[('gnw', l)]:
                                               OFFV[('gnw', l)] + CH])
                        gnb_r = vp.tile([128, CH], bf, tag="gnb_r")
                        nc.sync.dma_start(
                            out=gnb_r, in_=vb_d[:, OFFV[('gnb', l)]:
                                               OFFV[('gnb', l)] + CH])

                    S_cur = sp.tile([128, CT, HS], bf, tag="S")
                    nc.vector.memset(S_cur, 0.0)

                    def tm_apply(pend, last=False):
                        """apply window w-1's AllReduced delta to the residual
                        stream; issued one window late so compute never head-
                        of-line blocks on the collective. The final apply runs
                        on the (otherwise idle) Pool queue so the next phase's
                        DVE/ACT work isn't queued behind the last AR."""
                        bout_p, sl_p = pend
                        add_eng = nc.gpsimd if last else nc.vector
                        ar = pb.tile([128, DT, W], bf, tag="ar")
                        nc.sync.dma_start(out=ar, in_=bout_p)
                        xnew = pa1.tile([128, DT, W], f32, tag="xnew")
                        if phase_tm == 0:
                            xb = pa1.tile([128, DT, W], bf, tag="xb")
                            nc.sync.dma_start(
                                out=xb, in_=x_d[:, :, sl_p]
                                .rearrange("j p w -> p j w"))
                            add_eng.tensor_add(xnew, xb, ar)
                        else:
                            nc.sync.dma_start(
                                out=xnew, in_=xres_t[:, :, sl_p]
                                .rearrange("j p w -> p j w"))
                            add_eng.tensor_add(xnew, xnew, ar)
                        xnb = pa1.tile([128, DT, W], bf, tag="xnb")
                        if last:
                            nc.gpsimd.tensor_copy(xnb, xnew)
                        else:
                            nc.scalar.activation(xnb, xnew, Act.Identity)
                        nc.sync.dma_start(
                            out=xres_t[:, :, sl_p].rearrange("j p w -> p j w"),
                            in_=xnew)
                        nc.sync.dma_start(
                            out=xresb_t[:, :, sl_p].rearrange("j p w -> p j w"),
                            in_=xnb)

                    pendq = []
                    xln_prev = None
                    for w in range(NW):
                        sl = slice(w * W, (w + 1) * W)
                        xwb = pw.tile([128, DT, W], bf, tag="xwb")
                        nc.sync.dma_start(out=xwb, in_=xsrc_bf(phase_tm, sl))
                        xln = layer_norm(pw, psA, ln1w, xwb)
                        xp_prev = xln_prev
                        xln_prev = xln
                        if _SUB < 2:
                            continue
                        sx = token_shift(pw, xln, xp_prev, w)

                        xxx = pw.tile([128, DT, W], bf, tag="xxx")
                        for j in range(DT):
                            nc.vector.scalar_tensor_tensor(
                                out=xxx[:, j, :], in0=sx[:, j, :],
                                scalar=maat["x_maa"][:, j:j + 1],
                                in1=xln[:, j, :], op0=Alu.mult, op1=Alu.add)

                        t320 = pw.tile([128, 2, W], bf, tag="t320")
                        for mt in range(2):
                            msz = min(128, 5 * E - mt * 128)
                            ps = psA.tile([128, W], f32, tag="mm")
                            mm_chain(ps[0:msz, :],
                                     lambda kt, mt=mt, msz=msz:
                                         tmw1[:, kt, mt * 128:mt * 128 + msz],
                                     lambda kt: xxx[:, kt, :], DT)
                            nc.scalar.activation(t320[0:msz, mt, :],
                                                 ps[0:msz, :], Act.Tanh)

                        bnames = ["w_maa", "k_maa", "v_maa", "r_maa", "g_maa"]
                        bx = {}
                        for f, n in enumerate(bnames):
                            bxf = pb.tile([128, DT, W], bf, tag=f"bx{f}")
                            poff = E * (f % 4)
                            fj = f // 4
                            s2a = pw.tile([128, DT, W], bf, tag="s2")
                            for mt in range(DT):
                                ps = psA.tile([128, W], f32, tag="mm")
                                nc.tensor.matmul(
                                    ps, lhsT=tmw2[poff:poff + E, fj,
                                                  mt * 128:(mt + 1) * 128],
                                    rhs=t320[poff:poff + E, fj, :],
                                    start=True, stop=True,
                                    tile_position=(poff, 0))
                                nc.vector.scalar_tensor_tensor(
                                    out=s2a[:, mt, :], in0=ps,
                                    scalar=maat[n][:, mt:mt + 1],
                                    in1=sx[:, mt, :], op0=Alu.add, op1=Alu.mult)
                            nc.vector.tensor_add(bxf, s2a, xln)
                            bx[n[0]] = bxf

                        if _SUB < 3:
                            continue
                        projs = {}
                        for n, wt, act in [("r", w_r, Act.Sigmoid),
                                           ("k", w_k, None), ("v", w_v, None),
                                           ("g", w_g, "silu")]:
                            out_t = pk.tile([128, CT, W], bf, tag=f"proj_{n}")
                            for mt in range(CT):
                                ps = psA.tile([128, W], f32, tag="mm")
                                mm_chain(ps,
                                         lambda kt, wt=wt, mt=mt:
                                             wt[:, kt, mt * 128:(mt + 1) * 128],
                                         lambda kt, n=n: bx[n][:, kt, :], DT)
                                if act is None:
                                    nc.vector.tensor_scalar_mul(
                                        out_t[:, mt, :], ps, ISCW)
                                elif act == "silu":
                                    sgt = pw.tile([128, W], bf, tag="sgt")
                                    nc.scalar.activation(sgt, ps, Act.Sigmoid,
                                                         scale=ISCW)
                                    nc.vector.tensor_mul(out_t[:, mt, :], sgt,
                                                         ps)
                                else:
                                    nc.scalar.activation(out_t[:, mt, :], ps,
                                                         act, scale=ISCW)
                            projs[n] = out_t

                        ps = psA.tile([128, W], f32, tag="mm")
                        mm_chain(ps[0:ED, :],
                                 lambda kt: tdw1[:, kt, :],
                                 lambda kt: bx["w"][:, kt, :], DT)
                        t64 = pw.tile([ED, W], bf, tag="t64")
                        nc.scalar.activation(t64, ps[0:ED, :], Act.Tanh)
                        wtotW = pw.tile([128, CT, NCH], f32, tag="wtotW")
                        rt_t = pk.tile([128, CT, W], bf, tag="rt_t")
                        kt_t = pk.tile([128, CT, W], bf, tag="kt_t")
                        kh_t = pk.tile([128, CT, W], bf, tag="kh_t")
                        rk_t = pk.tile([128, CT, W], bf, tag="rk_t")
                        e_all = pk.tile([128, CT, W], f32, tag="e_all")
                        P_all = pk.tile([128, CT, W], f32, tag="P_all")
                        for mt in range(CT):
                            ps2 = psA.tile([128, W], f32, tag="mm")
                            nc.tensor.matmul(ps2,
                                             lhsT=tdw2[:, mt * 128:(mt + 1) * 128],
                                             rhs=t64, start=True, stop=True)
                            nc.scalar.activation(e_all[:, mt, :], ps2, Act.Exp,
                                                 bias=tdb[:, mt:mt + 1],
                                                 scale=1.0)
                            # P = cumsum(e) within each chunk;  c_incl = -P
                            for c in range(NCH):
                                csl = slice(c * CK, (c + 1) * CK)
                                nc.vector.tensor_tensor_scan(
                                    out=P_all[:, mt, csl],
                                    data0=e_all[:, mt, csl],
                                    data1=zerosCK, initial=0.0,
                                    op0=Alu.add, op1=Alu.add)
                            nc.vector.scalar_tensor_tensor(
                                out=rk_t[:, mt, :], in0=projs["r"][:, mt, :],
                                scalar=ut[:, mt:mt + 1],
                                in1=projs["k"][:, mt, :],
                                op0=Alu.mult, op1=Alu.mult)
                        # batched decay tail over all mt at once
                        nc.vector.tensor_sub(e_all, P_all, e_all)  # := P - e
                        expA = pk.tile([128, CT, W], bf, tag="expA")
                        nc.scalar.activation(expA, e_all, Act.Exp, scale=-1.0)
                        expB = pk.tile([128, CT, W], bf, tag="expB")
                        nc.scalar.activation(expB, P_all, Act.Exp, scale=1.0)
                        pvv = P_all.rearrange("p m (c u) -> p (m c) u", u=CK)
                        nc.scalar.activation(
                            wtotW.rearrange("p m (c u) -> p (m c) u", u=1),
                            pvv[:, :, CK - 1:CK], Act.Exp, scale=-1.0)
                        nc.vector.tensor_mul(rt_t, projs["r"], expA)
                        nc.vector.tensor_mul(kt_t, projs["k"], expB)
                        for mt in range(CT):
                            for c in range(NCH):
                                csl = slice(c * CK, (c + 1) * CK)
                                nc.scalar.activation(
                                    kh_t[:, mt, csl], kt_t[:, mt, csl],
                                    Act.Identity,
                                    scale=wtotW[:, mt, c:c + 1])

                        if _SUB < 4:
                            continue
                        ygT = pw.tile([128, CT, W], bf, tag="ygT")
                        for c in range(NCH):
                            csl = slice(c * CK, (c + 1) * CK)
                            VT = pw.tile([128, CH], bf, tag="VT")
                            KhT = pw.tile([128, CH], bf, tag="KhT")
                            for jt in range(CT):
                                pt = psT.tile([128, 128], bf, tag="tr")
                                nc.tensor.transpose(pt, projs["v"][:, jt, csl],
                                                    ident)
                                nc.scalar.activation(
                                    VT[:, 128 * jt:128 * (jt + 1)], pt,
                                    Act.Identity)
                                pt2 = psT.tile([128, 128], bf, tag="tr")
                                nc.tensor.transpose(pt2, kh_t[:, jt, csl], ident)
                                nc.scalar.activation(
                                    KhT[:, 128 * jt:128 * (jt + 1)], pt2,
                                    Act.Identity)

                            if _SUB < 5:
                                continue
                            # d_i = sum_k r*u*k per head -> dT [tok, 8]
                            ps_d = psB.tile([128, CH], f32, tag="wkvE")
                            for jt in range(CT):
                                nc.tensor.matmul(
                                    ps_d[0:2, 128 * jt:128 * (jt + 1)],
                                    lhsT=blkdiag, rhs=rk_t[:, jt, csl],
                                    start=True, stop=True)
                            d8 = pw.tile([128, CK], bf, tag="d8")
                            for jt in range(CT):
                                nc.vector.tensor_copy(
                                    d8[32 * jt:32 * jt + 2, :],
                                    ps_d[0:2, 128 * jt:128 * (jt + 1)])
                            ptd = psT.tile([128, 128], bf, tag="tr")
                            nc.tensor.transpose(ptd, d8, ident)
                            dT = pw.tile([128, CT, 2], bf, tag="dT")
                            nc.vector.tensor_copy(
                                dT, ptd.rearrange("p (a b) -> p a b",
                                                  b=32)[:, :, 0:2])

                            if _SUB < 6:
                                continue
                            psO = psB.tile([128, CH], f32, tag="wkvO")
                            psE = psB.tile([128, CH], f32, tag="wkvE")
                            psD = psB.tile([128, CH], f32, tag="wkvD")
                            Am = pw.tile([128, CH], bf, tag="Am")
                            Am2 = pw.tile([128, CH], bf, tag="Am2")
                            for h in range(NH):
                                poff = HS * (h % 2)
                                jt = h // 2
                                bank = psE if h % 2 == 0 else psD
                                nc.tensor.matmul(
                                    bank[:, 128 * (h // 2):128 * (h // 2 + 1)],
                                    lhsT=kt_t[poff:poff + HS, jt, csl],
                                    rhs=rt_t[poff:poff + HS, jt, csl],
                                    start=True, stop=True,
                                    tile_position=(poff, 0))
                            for h in range(NH):
                                bank = psE if h % 2 == 0 else psD
                                am = Am if h % 2 == 0 else Am2
                                nc.vector.tensor_mul(
                                    am[:, 128 * (h // 2):128 * (h // 2 + 1)],
                                    bank[:, 128 * (h // 2):128 * (h // 2 + 1)],
                                    maskstr)
                            for h in range(NH):
                                poff = HS * (h % 2)
                                jt = h // 2
                                am = Am if h % 2 == 0 else Am2
                                nc.tensor.matmul(
                                    psO[:, HS * h:HS * (h + 1)],
                                    lhsT=am[:, 128 * (h // 2):128 * (h // 2 + 1)],
                                    rhs=VT[:, HS * h:HS * (h + 1)],
                                    start=True, stop=False)
                                nc.tensor.matmul(
                                    psO[:, HS * h:HS * (h + 1)],
                                    lhsT=rt_t[poff:poff + HS, jt, csl],
                                    rhs=S_cur[poff:poff + HS, jt, :],
                                    start=False, stop=True,
                                    tile_position=(poff, 0))

                            O_sb = pw.tile([128, CH], bf, tag="O_sb")
                            for h in range(NH):
                                nc.vector.scalar_tensor_tensor(
                                    out=O_sb[:, HS * h:HS * (h + 1)],
                                    in0=VT[:, HS * h:HS * (h + 1)],
                                    scalar=dT[:, h // 2, (h % 2):(h % 2) + 1],
                                    in1=psO[:, HS * h:HS * (h + 1)],
                                    op0=Alu.mult, op1=Alu.add)

                            if _SUB < 7:
                                continue
                            psSe = psB.tile([128, CT, HS], f32, tag="wkvSE")
                            psSd = psB.tile([128, CT, HS], f32, tag="wkvSD")
                            for h in range(NH):
                                poff = HS * (h % 2)
                                jt = h // 2
                                bank = psSe if h % 2 == 0 else psSd
                                nc.tensor.matmul(
                                    bank[poff:poff + HS, jt, :],
                                    lhsT=KhT[:, HS * h:HS * (h + 1)],
                                    rhs=VT[:, HS * h:HS * (h + 1)],
                                    start=True, stop=True,
                                    tile_position=(0, poff))
                            S_new = sp.tile([128, CT, HS], bf, tag="S")
                            for jt in range(CT):
                                nc.vector.scalar_tensor_tensor(
                                    out=S_new[0:HS, jt, :],
                                    in0=S_cur[0:HS, jt, :],
                                    scalar=wtotW[0:HS, jt, c:c + 1],
                                    in1=psSe[0:HS, jt, :],
                                    op0=Alu.mult, op1=Alu.add)
                                nc.vector.scalar_tensor_tensor(
                                    out=S_new[HS:128, jt, :],
                                    in0=S_cur[HS:128, jt, :],
                                    scalar=wtotW[HS:128, jt, c:c + 1],
                                    in1=psSd[HS:128, jt, :],
                                    op0=Alu.mult, op1=Alu.add)
                            S_cur = S_new

                            if _SUB < 8:
                                continue
                            # GroupNorm per head
                            stt = pw.tile([128, NH, 6], f32, tag="gn_st")
                            mv = pw.tile([128, NH, 2], f32, tag="gn_mv")
                            for h in range(NH):
                                nc.vector.bn_stats(stt[:, h, :],
                                                   O_sb[:, HS * h:HS * (h + 1)])
                                nc.vector.bn_aggr(mv[:, h, :], stt[:, h, :])
                            sd8 = pw.tile([128, NH], f32, tag="gn_sd")
                            nc.scalar.activation(
                                sd8.rearrange("p (c u) -> p c u", u=1),
                                mv[:, :, 1:2], Act.Sqrt, bias=epst, scale=1.0)
                            rs8 = pw.tile([128, NH], f32, tag="gn_rs")
                            nc.vector.reciprocal(rs8, sd8)
                            nm8 = pw.tile([128, NH], f32, tag="gn_nm")
                            nc.vector.tensor_mul(
                                nm8.rearrange("p (c u) -> p c u", u=1),
                                mv[:, :, 0:1],
                                rs8.rearrange("p (c u) -> p c u", u=1))
                            nc.vector.tensor_scalar_mul(nm8, nm8, -1.0)
                            Ogn = pw.tile([128, CH], bf, tag="Ogn")
                            for h in range(NH):
                                nc.scalar.activation(
                                    Ogn[:, HS * h:HS * (h + 1)],
                                    O_sb[:, HS * h:HS * (h + 1)], Act.Identity,
                                    bias=nm8[:, h:h + 1], scale=rs8[:, h:h + 1])
                            if not skip_gn_affine:
                                t1g = pw.tile([128, CH], bf, tag="gn_t1")
                                nc.vector.tensor_mul(t1g, Ogn, gnw_r)
                                nc.vector.tensor_add(Ogn, t1g, gnb_r)

                            for jt in range(CT):
                                pt3 = psT.tile([128, 128], bf, tag="tr")
                                nc.tensor.transpose(
                                    pt3, Ogn[:, 128 * jt:128 * (jt + 1)], ident)
                                nc.vector.scalar_tensor_tensor(
                                    out=ygT[:, jt, csl], in0=pt3, scalar=ISCW,
                                    in1=projs["g"][:, jt, csl],
                                    op0=Alu.mult, op1=Alu.mult)

                        if _SUB < 10:
                            continue
                        delta = pb.tile([128, DT, W], bf, tag="delta")
                        for mt in range(DT):
                            ps = psA.tile([128, W], f32, tag="mm")
                            mm_chain(ps,
                                     lambda kt, mt=mt:
                                         wo[:, kt, mt * 128:(mt + 1) * 128],
                                     lambda kt: ygT[:, kt, :], CT)
                            nc.scalar.activation(delta[:, mt, :], ps,
                                                 Act.Identity, scale=ISCW)

                        # bf16 payload: halves NeuronLink bytes on the AR
                        bin_ = dram.tile([128, DT, W], bf, tag="bin")
                        bout = dram.tile([128, DT, W], bf, tag="bout")
                        nc.sync.dma_start(out=bin_, in_=delta)
                        nc.gpsimd.collective_compute(
                            "AllReduce", Alu.add, replica_groups=PAIRS,
                            ins=[bin_.opt()], outs=[bout.opt()])
                        # 2-deep lag: the in-order collective queue drifts,
                        # so give each AR two windows of compute to hide under
                        if len(pendq) >= 2:
                            tm_apply(pendq.pop(0))
                        pendq.append((bout, sl))
                    if pendq:
                        for p in pendq[:-1]:
                            tm_apply(p)
                        tm_apply(pendq[-1], last=True)

                # ---------------- channel mix ----------------
                if _STOP < 2 + 2 * l:
                    continue
                with tc.tile_pool(name=f"wcm{l}", bufs=1) as wp2, \
                     tc.tile_pool(name=f"vc2{l}", bufs=1) as vp2, \
                     tc.tile_pool(name=f"cma{l}", bufs=2) as pw2, \
                     tc.tile_pool(name=f"cap{l}", bufs=1) as pa2, \
                     tc.tile_pool(name=f"psc{l}", bufs=4, space="PSUM") as psC:

                    # chunked loads (one DMA per 512-col group) so the first
                    # mt-tile matmuls start before the full 13MB lands
                    cwk = wp2.tile([128, DT, FT * 128], bf, tag="cwk")
                    cwr = wp2.tile([128, DT, D], bf, tag="cwr")
                    cwv = wp2.tile([128, FT, D], bf, tag="cwv")
                    for q in range(3):
                        nc.sync.dma_start(
                            out=cwk[:, :, 512 * q:512 * (q + 1)],
                            in_=wag_out[OFF['cWk'] + q:OFF['cWk'] + 3 * DT:3]
                            .rearrange("j p c -> p j c"))
                    for q in range(2):
                        nc.sync.dma_start(
                            out=cwr[:, :, 512 * q:512 * (q + 1)],
                            in_=wag_out[OFF['cWr'] + q:OFF['cWr'] + 2 * DT:2]
                            .rearrange("j p c -> p j c"))
                        nc.sync.dma_start(
                            out=cwv[:, :, 512 * q:512 * (q + 1)],
                            in_=wag_out[OFF['cWv'] + q:OFF['cWv'] + 2 * FT:2]
                            .rearrange("j p c -> p j c"))
                    if skip_ln_w:
                        ln2w = None
                    else:
                        ln2w = vp2.tile([128, DT], f32, tag="ln2w")
                        nc.sync.dma_start(
                            out=ln2w, in_=vb_d[:, OFFV[('ln2', l)]:
                                              OFFV[('ln2', l)] + DT])
                    ckm = vp2.tile([128, DT], f32, tag="ckm")
                    nc.sync.dma_start(
                        out=ckm, in_=vb_d[:, OFFV[('ck_maa', l)]:
                                          OFFV[('ck_maa', l)] + DT])
                    crm = vp2.tile([128, DT], f32, tag="crm")
                    nc.sync.dma_start(
                        out=crm, in_=vb_d[:, OFFV[('cr_maa', l)]:
                                          OFFV[('cr_maa', l)] + DT])

                    def cm_apply(pend, last=False):
                        bout_p, sl_p, rr_p = pend
                        eng = nc.gpsimd if last else nc.vector
                        ar2 = pa2.tile([128, DT, W], bf, tag="ar2")
                        nc.sync.dma_start(out=ar2, in_=bout_p)
                        xwf2 = pa2.tile([128, DT, W], f32, tag="xwf2")
                        nc.sync.dma_start(
                            out=xwf2, in_=xres_t[:, :, sl_p]
                            .rearrange("j p w -> p j w"))
                        tmp2 = pa2.tile([128, DT, W], f32, tag="tmp2")
                        eng.tensor_mul(tmp2, rr_p, ar2)
                        eng.tensor_add(xwf2, xwf2, tmp2)
                        xnb2 = pa2.tile([128, DT, W], bf, tag="xnb2")
                        if last:
                            nc.gpsimd.tensor_copy(xnb2, xwf2)
                        else:
                            nc.scalar.activation(xnb2, xwf2, Act.Identity)
                        nc.sync.dma_start(
                            out=xres_t[:, :, sl_p].rearrange("j p w -> p j w"),
                            in_=xwf2)
                        nc.sync.dma_start(
                            out=xresb_t[:, :, sl_p].rearrange("j p w -> p j w"),
                            in_=xnb2)

                    pendq = []
                    xln2_prev = None
                    for w in range(NW):
                        sl = slice(w * W, (w + 1) * W)
                        xwb2 = pw2.tile([128, DT, W], bf, tag="xwb2")
                        nc.sync.dma_start(
                            out=xwb2, in_=xresb_t[:, :, sl]
                            .rearrange("j p w -> p j w"))
                        xln2 = layer_norm(pw2, psC, ln2w, xwb2)
                        sx2 = token_shift(pw2, xln2, xln2_prev, w)
                        kx = pw2.tile([128, DT, W], bf, tag="kx")
                        rx = pw2.tile([128, DT, W], bf, tag="rx")
                        for j in range(DT):
                            nc.vector.scalar_tensor_tensor(
                                out=kx[:, j, :], in0=sx2[:, j, :],
                                scalar=ckm[:, j:j + 1], in1=xln2[:, j, :],
                                op0=Alu.mult, op1=Alu.add)
                            nc.vector.scalar_tensor_tensor(
                                out=rx[:, j, :], in0=sx2[:, j, :],
                                scalar=crm[:, j:j + 1], in1=xln2[:, j, :],
                                op0=Alu.mult, op1=Alu.add)

                        kk = pw2.tile([128, FT, W], bf, tag="kk")
                        for mt in range(FT):
                            ps = psC.tile([128, W], f32, tag="mm")
                            mm_chain(ps,
                                     lambda kt, mt=mt:
                                         cwk[:, kt, mt * 128:(mt + 1) * 128],
                                     lambda kt: kx[:, kt, :], DT)
                            kk0 = pw2.tile([128, W], bf, tag="kk0")
                            nc.scalar.activation(kk0, ps, Act.Relu, scale=ISCW)
                            nc.scalar.activation(kk[:, mt, :], kk0, Act.Square)

                        delta2 = pw2.tile([128, DT, W], bf, tag="delta2")
                        for mt in range(DT):
                            ps = psC.tile([128, W], f32, tag="mm")
                            mm_chain(ps,
                                     lambda kt, mt=mt:
                                         cwv[:, kt, mt * 128:(mt + 1) * 128],
                                     lambda kt: kk[:, kt, :], FT)
                            nc.scalar.activation(delta2[:, mt, :], ps,
                                                 Act.Identity, scale=ISCW)

                        rr = pw2.tile([128, DT, W], bf, tag="rr", bufs=3)
                        for mt in range(DT):
                            ps = psC.tile([128, W], f32, tag="mm")
                            mm_chain(ps,
                                     lambda kt, mt=mt:
                                         cwr[:, kt, mt * 128:(mt + 1) * 128],
                                     lambda kt: rx[:, kt, :], DT)
                            nc.scalar.activation(rr[:, mt, :], ps, Act.Sigmoid,
                                                 scale=ISCW)

                        bin2 = dram.tile([128, DT, W], bf, tag="bin")
                        bout2 = dram.tile([128, DT, W], bf, tag="bout")
                        nc.sync.dma_start(out=bin2, in_=delta2)
                        nc.gpsimd.collective_compute(
                            "AllReduce", Alu.add, replica_groups=PAIRS,
                            ins=[bin2.opt()], outs=[bout2.opt()])
                        if len(pendq) >= 2:
                            cm_apply(pendq.pop(0))
                        pendq.append((bout2, sl, rr))
                        xln2_prev = xln2
                    if pendq:
                        for p in pendq[:-1]:
                            cm_apply(p)
                        cm_apply(pendq[-1], last=True)

            # ---------------- final LN ----------------
            with tc.tile_pool(name="fin", bufs=2) as pf, \
                 tc.tile_pool(name="psf", bufs=2, space="PSUM") as psF, \
                 tc.tile_pool(name="vecf", bufs=1) as vf:
                if skip_ln_w:
                    lnfw = None
                else:
                    lnfw = vf.tile([128, DT], f32, tag="lnfw")
                    nc.sync.dma_start(
                        out=lnfw,
                        in_=vb_d[:, OFFV['lnf']:OFFV['lnf'] + DT])
                for w in range(NWC):
                    sl = slice(w * WC, (w + 1) * WC)
                    xwb3 = pf.tile([128, DT, WC], bf, tag="xwb3")
                    nc.sync.dma_start(out=xwb3, in_=xsrc_bf(2 * L, sl))
                    yw = layer_norm(pf, psF, lnfw, xwb3)
                    for jl in range(DT):
                        nc.sync.dma_start(out=y_d[jl, :, sl],
                                          in_=yw[:, jl, :])
    if split_waits:
        _split_sync_waits(nc, scratch=wsplit_sem)
    return nc


# ===================== host side =====================

_CACHE = {}
_RUNNER_CACHE = {}


class _Runner:
    """AOT-compiled PJRT executor for a built Bass module.

    Mirrors bass2jax.run_bass_via_pjrt's lowering exactly, but (a) traces,
    lowers and XLA-compiles ONCE, and (b) takes device-resident global
    arrays, so a steady-state call is just dispatch + NEFF execution —
    no re-trace, no re-compile, no host->device weight re-upload.
    """

    def __init__(self, nc, n_cores=8):
        import jax
        import jax.numpy as jnp
        from jax.sharding import Mesh, NamedSharding, PartitionSpec
        from jax.experimental.shard_map import shard_map
        from concourse import bass2jax

        bass2jax.install_neuronx_cc_hook()
        self.nc = nc
        self.n_cores = n_cores
        self._jax = jax

        partition_name = (nc.partition_id_tensor.name
                          if nc.partition_id_tensor else None)
        in_names, out_names, out_avals, zero_specs = [], [], [], []
        for alloc in nc.m.functions[0].allocations:
            if not isinstance(alloc, mybir.MemoryLocationSet):
                continue
            name = alloc.memorylocations[0].name
            if alloc.kind == "ExternalInput":
                if name != partition_name:
                    in_names.append(name)
            elif alloc.kind == "ExternalOutput":
                assert alloc.tensor_shape is not None
                shape = tuple(alloc.tensor_shape)
                dtype = mybir.dt.np(alloc.dtype)
                out_names.append(name)
                out_avals.append(jax.core.ShapedArray(shape, dtype))
                zero_specs.append((shape, dtype))

        self.dbg_name = None
        if nc.dbg_addr is not None:
            if nc.dbg_callbacks:
                raise RuntimeError("dbg_callbacks unsupported under axon")
            self.dbg_name = nc.dbg_addr.name

        n_params = len(in_names)
        self.param_names = list(in_names)
        self.out_names = list(out_names)
        self.out_avals = out_avals
        in_names_full = list(in_names) + list(out_names)
        if partition_name is not None:
            in_names_full.append(partition_name)
        donate = tuple(range(n_params, n_params + len(out_names)))

        def _body(*args):
            operands = list(args)
            if partition_name is not None:
                operands.append(bass2jax.partition_id_tensor())
            outs = bass2jax._bass_exec_p.bind(
                *operands,
                out_avals=tuple(out_avals),
                in_names=tuple(in_names_full),
                out_names=tuple(out_names),
                lowering_input_output_aliases=(),
                sim_require_finite=True,
                sim_require_nnan=True,
                nc=nc,
            )
            return tuple(outs)

        devices = jax.devices()[:n_cores]
        assert len(devices) == n_cores
        mesh = Mesh(np.asarray(devices), ("core",))
        self.sharding = NamedSharding(mesh, PartitionSpec("core"))
        in_specs = (PartitionSpec("core"),) * (n_params + len(out_names))
        out_specs = (PartitionSpec("core"),) * len(out_names)

        # global (concatenated) avals for AOT lowering
        in_sds = []
        for nm in in_names:
            shape, dtype = self._global_spec(nm)
            in_sds.append(jax.ShapeDtypeStruct(shape, dtype))
        zero_sds = [
            jax.ShapeDtypeStruct((n_cores * s[0],) + tuple(s[1:]), d)
            for (s, d) in zero_specs
        ]

        def _compile():
            jitted = jax.jit(
                shard_map(_body, mesh=mesh, in_specs=in_specs,
                          out_specs=out_specs, check_rep=False),
                donate_argnums=donate, keep_unused=True)
            return jitted.lower(*in_sds, *zero_sds).compile()

        try:
            self.compiled = bass2jax.fast_dispatch_compile(_compile)
        except Exception:
            self.compiled = _compile()

        self._make_zeros = jax.jit(
            lambda: tuple(jnp.zeros(s.shape, s.dtype) for s in zero_sds),
            out_shardings=tuple(self.sharding for _ in zero_sds))

    def _global_spec(self, name):
        """global concat shape/dtype for an ExternalInput name"""
        for alloc in self.nc.m.functions[0].allocations:
            if (isinstance(alloc, mybir.MemoryLocationSet)
                    and alloc.memorylocations[0].name == name):
                shape = tuple(alloc.tensor_shape)
                if name == self.dbg_name:
                    # uint32[1,2] view of the 8-byte dbg PA (x64 off)
                    return (self.n_cores, 2), np.uint32
                return ((self.n_cores * shape[0],) + shape[1:],
                        mybir.dt.np(alloc.dtype))
        raise KeyError(name)

    def put(self, conc):
        """device_put global arrays (dict name->np) -> resident arg list"""
        jax = self._jax
        args = []
        for nm in self.param_names:
            if nm == self.dbg_name and nm not in conc:
                a = np.zeros((self.n_cores, 2), np.uint32)
            else:
                a = conc[nm]
            args.append(jax.device_put(a, self.sharding))
        return args

    def run(self, dev_args):
        """execute; returns tuple of global sharded output arrays"""
        return self.compiled(*dev_args, *self._make_zeros())


def _get_runner(T, skip_gn_affine, skip_ln_w):
    key = (T, skip_gn_affine, skip_ln_w)
    if key not in _RUNNER_CACHE:
        _RUNNER_CACHE[key] = _Runner(_get_nc(T, skip_gn_affine, skip_ln_w))
    return _RUNNER_CACHE[key]


def _concat_inputs(inputs, T):
    in_maps = [_prep_core_inputs(inputs, c, T) for c in range(8)]
    return {n: np.concatenate([m[n] for m in in_maps], axis=0)
            for n in in_maps[0]}


def _prep_core_inputs(inputs, core, T):
    b, half = core // 2, core % 2
    hsl = slice(half * CH, (half + 1) * CH)
    fsl = slice(half * FT * 128, (half + 1) * FT * 128)

    def kt_tiles(w):
        return np.ascontiguousarray(
            w.reshape(w.shape[0] // 128, 128, w.shape[1]).astype(BF16))

    def vec_tiles(v):
        return np.ascontiguousarray(v.reshape(-1, 128).T.astype(np.float32))

    def bricks(w):
        """[n,128,C] f32-ish, C % 512 == 0 -> [n*C/512, 128, 512] fp8*SCW"""
        n, p, cc = w.shape
        assert p == 128 and cc % 512 == 0
        return np.ascontiguousarray(
            (w.astype(np.float32) * SCW)
            .reshape(n, 128, cc // 512, 512).transpose(0, 2, 1, 3)
            .reshape(-1, 128, 512).astype(BF16))

    # x ships full per core (device-resident across calls)
    xt = inputs["x"][b][:T].T.astype(BF16)        # [D, T]
    xt = xt.reshape(DT, 128, T)
    out = {"x": np.ascontiguousarray(xt)}

    # big-weight blob for this core's half, 4-way sharded by batch index
    blob = np.zeros((NBR, 128, 512), BF16)
    for l in range(L):
        OFF = _woff(l)
        blob[OFF['Wr']:OFF['Wr'] + 8] = bricks(kt_tiles(inputs["Wr"][l][:, hsl]))
        blob[OFF['Wk']:OFF['Wk'] + 8] = bricks(kt_tiles(inputs["Wk"][l][:, hsl]))
        blob[OFF['Wv']:OFF['Wv'] + 8] = bricks(kt_tiles(inputs["Wv"][l][:, hsl]))
        blob[OFF['Wg']:OFF['Wg'] + 8] = bricks(kt_tiles(inputs["Wg"][l][:, hsl]))
        blob[OFF['Wo']:OFF['Wo'] + 8] = bricks(kt_tiles(inputs["Wo"][l][hsl, :]))
        blob[OFF['cWk']:OFF['cWk'] + 24] = bricks(kt_tiles(inputs["cWk"][l][:, fsl]))
        blob[OFF['cWv']:OFF['cWv'] + 24] = bricks(kt_tiles(inputs["cWv"][l][fsl, :]))
        blob[OFF['cWr']:OFF['cWr'] + 16] = bricks(kt_tiles(inputs["cWr"][l]))
        # small weights, packed into spare bricks
        tm1 = kt_tiles(inputs["tm_w1"][l])          # [8, 128, 160]
        for t in range(DT):
            blob[OFF['tm_w1'] + t // 3, :, 160 * (t % 3):160 * (t % 3) + 160] \
                = tm1[t]
        tw2 = np.zeros((2, 128, D), np.float32)
        for f in range(5):
            tw2[f // 4, E * (f % 4):E * (f % 4) + E] = inputs["tm_w2"][l, f]
        for j in range(2):
            for q in range(2):
                blob[OFF['tm_w2'] + 2 * j + q] = \
                    tw2[j, :, 512 * q:512 * (q + 1)].astype(BF16)
        td1 = kt_tiles(inputs["td_w1"][l])          # [8, 128, 64]
        for t in range(DT):
            blob[OFF['td_w1'], :, 64 * t:64 * (t + 1)] = td1[t]
        blob[OFF['td_w2'], 0:ED, :] = inputs["td_w2"][l][:, hsl].astype(BF16)
    # extra brick: ident + blkdiag constants
    extra = np.zeros((1, 128, 512), BF16)
    extra[0, :, 0:128] = np.eye(128, dtype=BF16)
    extra[0, 0:64, 128] = 1
    extra[0, 64:128, 129] = 1
    out["wblob"] = np.ascontiguousarray(
        np.concatenate([blob, extra], axis=0))

    skip_gn, skip_ln = _flags(inputs)
    OFFV, VC = _vlayout(skip_gn, skip_ln)
    vb = np.zeros((128, VC), np.float32)
    vb[:, OFFV['maskstr']:OFFV['maskstr'] + 128] = \
        np.triu(np.ones((128, 128), np.float32), 1)
    for l in range(L):
        for n in ["x_maa", "w_maa", "k_maa", "v_maa", "r_maa", "g_maa",
                  "ck_maa", "cr_maa"]:
            vb[:, OFFV[(n, l)]:OFFV[(n, l)] + DT] = vec_tiles(inputs[n][l])
        vb[:, OFFV[('tdb', l)]:OFFV[('tdb', l)] + CT] = \
            vec_tiles(inputs["time_decay"][l].reshape(-1)[hsl])
        vb[:, OFFV[('u', l)]:OFFV[('u', l)] + CT] = \
            vec_tiles(inputs["time_first"][l].reshape(-1)[hsl])
        if not skip_ln:
            vb[:, OFFV[('ln1', l)]:OFFV[('ln1', l)] + DT] = \
                vec_tiles(inputs["ln1_w"][l])
            vb[:, OFFV[('ln2', l)]:OFFV[('ln2', l)] + DT] = \
                vec_tiles(inputs["ln2_w"][l])
        if not skip_gn:
            vb[:, OFFV[('gnw', l)]:OFFV[('gnw', l)] + CH] = \
                inputs["gn_w"][l][hsl][None, :]
            vb[:, OFFV[('gnb', l)]:OFFV[('gnb', l)] + CH] = \
                inputs["gn_b"][l][hsl][None, :]
    if not skip_ln:
        vb[:, OFFV['lnf']:OFFV['lnf'] + DT] = vec_tiles(inputs["ln_out_w"])
    out["vblob"] = vb
    return out


def _get_nc(T, skip_gn_affine, skip_ln_w):
    key = (T, skip_gn_affine, skip_ln_w)
    if key not in _CACHE:
        nc = bass.Bass(trn_type="TRN2", num_devices=8)
        build(nc, T, skip_gn_affine, skip_ln_w)
        # the module is immutable after build; memoize its (large) JSON
        # serialization so repeated run_bass_kernel_spmd calls don't redo it
        raw = nc.to_json_bytes()
        nc.to_json_bytes = lambda: raw
        _CACHE[key] = nc
    return _CACHE[key]


def _flags(inputs):
    skip_gn = bool(np.all(inputs["gn_w"] == 1.0)
                   and np.all(inputs["gn_b"] == 0.0))
    skip_ln = bool(np.all(inputs["ln1_w"] == 1.0)
                   and np.all(inputs["ln2_w"] == 1.0)
                   and np.all(inputs["ln_out_w"] == 1.0))
    return skip_gn, skip_ln


def _gather_output(y_global, T):
    """y_global: [8*DT, 128, T] -> full [B, T, D] f32.
    Both cores of a pair compute the full (identical) y; use the even one."""
    y = np.asarray(y_global).reshape(8, DT, 128, T)
    outs = [y[2 * b].transpose(2, 0, 1).reshape(T, D) for b in range(B)]
    return np.stack(outs).astype(np.float32)


def kernel(**inputs):
    inputs = {k: np.asarray(v) for k, v in inputs.items()}
    T = inputs["x"].shape[1]
    skip_gn, skip_ln = _flags(inputs)
    runner = _get_runner(T, skip_gn, skip_ln)
    conc = _concat_inputs(inputs, T)
    dev = runner.put(conc)
    outs = runner.run(dev)
    return _gather_output(outs[0], T)


if __name__ == "__main__":
    import sys
    Tk = int(sys.argv[1]) if len(sys.argv) > 1 else 2048
    z = np.load('/tmp/inputs.npz')
    inputs = {k: z[k] for k in z.files}
    inputs["x"] = np.ascontiguousarray(inputs["x"][:, :Tk])
    act = kernel(**inputs)
    import np_ref
    exp = np_ref.np_reference(inputs)
    rel = np.linalg.norm(act - exp) / np.linalg.norm(exp)
    print("rel l2 vs np reference:", rel)
    print("max abs diff:", np.abs(act - exp).max())

